# revision 1
# baseline (speedup 1.0000x reference)
"""Trainium2 Bass kernel for nn_Locally_Connected_Module.

Network: 3 locally-connected 3x3 layers (per-location weights, ~57MB total),
then 4 conv3x3+BN(+PReLU/tanh) blocks with 3 maxpools.
  x (32,3,32,32) -> LC1 -> (32,32,30,30) -> LC2 -> (32,32,28,28) -> LC3 ->
  (32,32,26,26) -> conv1+bn+prelu+pool -> (32,64,13,13) -> conv2.. ->
  (32,128,6,6) -> conv3.. -> (32,256,3,3) -> convf+bn+tanh -> (32,256,3,3)

Sharding:
  Stage A (LC layers): SPATIAL row-sharding over the 8 cores; each core
  computes a 4-row slice of LC3 output (with halo back through LC2/LC1) for
  the FULL batch, so each core only reads ~1/8 of the per-location LC
  weights. LC bias is folded in as a K=97th "ones" row.
  Transition: AllToAll converts (all batch, row slice) -> (4 images,
  all rows) per core.
  Stage B (convs): batch-parallel, 4 images/core. Train-mode BN batch stats
  are reduced cross-core with small AllGathers (partial sum/sumsq per core,
  summed locally after the gather). Conv biases are skipped: train-mode BN
  makes them no-ops. Final output is batch-sharded; host concatenates.

Compute dtype: fp16 operands (PE matmul is 4x faster than fp32 and fp16
keeps ~11 mantissa bits vs bf16's 8, landing at ~7e-3 max rel err) with
fp32 PSUM accumulation and fp32 BN statistics math. Memsets only touch pad
borders; patch-shift replicas are width-trimmed to the consumed columns.

Experimental (env-gated OFF by default): KERNEL_RDMA/KERNEL_A2A switch the
BN-stat reductions and the AllToAll to XOR-slot remote_dma_broadcast
exchanges (~2us vs the ~15us fixed cost of each collective). The stats
exchange and the A2A each verify standalone (see probe_rdma*.py: XOR-slot
routing, symbolic row-offset out_ap, register-valued remote-sem waits),
but the combined full kernel still hits an opaque device fault (ruled
out: tile-pool reuse racing remote writes -- receive buffers are raw SBUF
tensors now; SWDGE ring overflow -- each broadcast prep is 66 descriptors
vs a 1024/queue ring, and raising dynamic_dma_scratch_size did not help).
collective_compute + remote_dma also cannot coexist in one NEFF (hangs),
so the verified collective path ships.
"""
import numpy as np

import concourse.bass as bass
import concourse.bacc as bacc
import concourse.mybir as mybir
import concourse.tile as tile
from concourse.bass_utils import run_bass_kernel_spmd

dt = mybir.dt
AF = mybir.ActivationFunctionType
ALU = mybir.AluOpType

import os
USE_FP16 = os.environ.get("KERNEL_FP16", "1") == "1"
# bitmask: bit (li-1) set -> BN layer li uses the remote_dma stat exchange;
# clear -> collective AllGather fallback
RDMA_MASK = int(os.environ.get("KERNEL_RDMA", "0"))
# AllToAll via remote_dma (Switch on the core's XOR label) vs collective.
# NOTE: collective_compute and remote_dma hang when mixed in one NEFF, so
# A2A=1 requires RDMA_MASK=15 and A2A=0 requires RDMA_MASK=0.
USE_RDMA_A2A = os.environ.get("KERNEL_A2A", "0") == "1"
# debug bitmask: bit (li-1) -> use local-only stats for BN layer li
LOCAL_STATS = int(os.environ.get("KERNEL_LOCALSTATS", "0"))
CAST = np.float16 if USE_FP16 else np.float32

NCORES = 8
CORES = list(range(NCORES))
EPS = 1e-5
ALPHA = 0.25

# LC3 output row starts per core (each computes rows [s, s+4) of 26)
ST = [0, 4, 8, 11, 14, 17, 20, 22]
# which global rows to take from each core's chunk when reassembling
TAKE = [(0, 4), (4, 8), (8, 12), (12, 15), (15, 18), (18, 21), (21, 24), (24, 26)]

N1, N2, N3, NF = 32 * 26 * 26, 32 * 13 * 13, 32 * 6 * 6, 32 * 3 * 3

_cache = {}


def _build_discovery():
    """Tiny NEFF: each core XOR-slot-broadcasts its logical id; receivers'
    slot k then holds the logical id of the core whose physical NC id is
    (own_pid ^ k). Core 0's slots give each logical core's XOR label d:
    d[slots0[k]] = k. Needed because the driver's logical->physical NC map
    is not readable from the client."""
    nc = bacc.Bacc("TRN2", target_bir_lowering=False)
    f32 = dt.float32
    val_d = nc.dram_tensor("val", [128, 1], f32, kind="ExternalInput")
    slots_d = nc.dram_tensor("slots", [128, 8], f32, kind="ExternalOutput")
    wreg = nc.vector.alloc_register("rwait")
    nc.vector.reg_mov(wreg, 16)
    with tile.TileContext(nc) as tc:
        with tc.tile_pool(name="p", bufs=1) as pool:
            VAL = pool.tile([128, 1], f32, tag="VAL")
            nc.sync.dma_start(VAL[:], val_d[:])
            RECV = pool.tile([128, 8], f32, tag="RECV")
            rsem = nc.alloc_semaphore("rsem")
            lsem = nc.alloc_semaphore("lsem")
            for k in range(8):
                rd = [None] * 8
                rd[k] = (0, k)
                nc.gpsimd.remote_dma_broadcast(
                    RECV[:, k:k + 1], VAL[:],
                    remote_sem=rsem, local_sem=lsem, rdests=rd)
            nc.gpsimd.trigger_dma(count=None)
            SL = pool.tile([128, 8], f32, tag="SL")
            cp = nc.vector.tensor_copy(SL[:], RECV[:])
            cp.wait_op(rsem, wreg, "sem-ge")
            nc.sync.dma_start(slots_d[:], SL[:])
    return nc


def _discover_dmap():
    """Run the discovery NEFF once; return d[logical] = XOR label."""
    if "dmap" not in _cache:
        nc = _build_discovery()
        nc.compile()
        in_maps = [{"val": np.full((128, 1), float(c), np.float32)}
                   for c in range(NCORES)]
        res = run_bass_kernel_spmd(nc, in_maps, CORES)
        slots0 = res.results[0]["slots"][0]  # [8] sender logical ids
        d = [-1] * NCORES
        for k in range(NCORES):
            d[int(round(float(slots0[k])))] = k
        assert sorted(d) == list(range(NCORES)), f"bad discovery: {slots0}"
        _cache["dmap"] = d
    return _cache["dmap"]


def _build():
    nc = bacc.Bacc("TRN2", target_bir_lowering=False, num_swdge_queues=4)
    f32 = dt.float32
    bf = dt.float16 if USE_FP16 else dt.float32

    # ---- external inputs (per-core data, same shapes on all cores) ----
    xp_d = nc.dram_tensor("xp", [28, 8, 32, 32], bf, kind="ExternalInput")
    w1p_d = nc.dram_tensor("w1p", [28, 8, 8, 128], bf, kind="ExternalInput")
    w2p_d = nc.dram_tensor("w2p", [6, 97, 7, 3, 128], bf, kind="ExternalInput")
    w3p_d = nc.dram_tensor("w3p", [4, 97, 7, 3, 128], bf, kind="ExternalInput")
    w1b_d = nc.dram_tensor("w1b", [96, 3, 64], bf, kind="ExternalInput")
    w2ba_d = nc.dram_tensor("w2ba", [128, 3, 128], bf, kind="ExternalInput")
    w2bb_d = nc.dram_tensor("w2bb", [64, 3, 128], bf, kind="ExternalInput")
    w3b_d = nc.dram_tensor("w3b", [128, 3, 3, 256], bf, kind="ExternalInput")
    wfb_d = nc.dram_tensor("wfb", [128, 2, 3, 3, 256], bf, kind="ExternalInput")
    bn1_d = nc.dram_tensor("bn1", [64, 2], f32, kind="ExternalInput")
    bn2_d = nc.dram_tensor("bn2", [128, 2], f32, kind="ExternalInput")
    bn3_d = nc.dram_tensor("bn3", [128, 2, 2], f32, kind="ExternalInput")
    bnf_d = nc.dram_tensor("bnf", [128, 2, 2], f32, kind="ExternalInput")

    out_d = nc.dram_tensor("out", [4, 256, 3, 3], f32, kind="ExternalOutput")
    # per-core row offset ST[label]*28 for the A2A receive window
    myrow_d = nc.dram_tensor("myrow", [1, 1], dt.uint32, kind="ExternalInput")

    # register-valued wait target for remote-sem gates (preamble block so it
    # is set before any tile-scheduled instruction decodes a wait against it)
    wreg = nc.vector.alloc_register("rdma_wait16")
    nc.vector.reg_mov(wreg, 16)
    rowreg = nc.gpsimd.alloc_register("myrow_reg")
    nc.gpsimd.reg_load(rowreg, myrow_d[0:1, 0:1])
    rowval = nc.gpsimd.snap(rowreg, donate=True, min_val=0, max_val=22 * 28)

    # remote-write targets live OUTSIDE the tile pools: peers' RDMA writes
    # land asynchronously and must never race tile-pool buffer reuse
    FDIM = {1: 2, 2: 2, 3: 4, 4: 4}  # free f32 elems per layer tag
    RCV = {li: nc.alloc_sbuf_tensor(f"RCV{li}", [128, 8, FDIM[li]], f32)
           for li in (1, 2, 3, 4)}
    PBQ2 = nc.alloc_sbuf_tensor("PBQ2", [128, 26, 28], bf)

    with tile.TileContext(nc) as tc:
        with (
            tc.tile_pool(name="const", bufs=1) as cpool,
            tc.tile_pool(name="wrow", bufs=3) as wpool,
            tc.tile_pool(name="act", bufs=1) as apool,
            tc.tile_pool(name="stat", bufs=1) as spool,
            tc.tile_pool(name="scr", bufs=2) as scrpool,
            tc.tile_pool(name="psum", bufs=4, space="PSUM") as pspool,
            tc.tile_pool(name="dram", bufs=1, space="DRAM") as dpool,
        ):
            # ---- BN-stat exchange buffers + preps (hoisted; data deps are
            # deferred to each queue's trigger) ----
            # queue map: A2A shares q0 with BNf (BNf preps are emitted after
            # the A2A trigger, so the q0 ring order stays A2A -> BNf)
            QMAP = {1: 1, 2: 2, 3: 3, 4: 0} if USE_RDMA_A2A \
                else {1: 0, 2: 1, 3: 2, 4: 3}
            SND, RSEM = {}, {}
            lsem = nc.alloc_semaphore("rdma_lsem")
            for li in (1, 2, 3, 4):
                SND[li] = spool.tile([128, FDIM[li]], f32, tag=f"SND{li}",
                                     name=f"SND{li}")
                RSEM[li] = nc.alloc_semaphore(f"rsem{li}")

            def emit_stat_preps(li):
                if (LOCAL_STATS >> (li - 1)) & 1 or not (RDMA_MASK >> (li - 1)) & 1:
                    return
                for k in range(8):
                    rd = [None] * 8
                    rd[k] = (0, k)
                    prep = nc.gpsimd.remote_dma_broadcast(
                        RCV[li][:, k, :], SND[li][:],
                        remote_sem=RSEM[li], local_sem=lsem,
                        rdests=rd, queue_num=QMAP[li])
                    if USE_RDMA_A2A and QMAP[li] == 0:
                        # q0 is shared with the A2A: descriptor generation
                        # must not enter the ring before the A2A trigger has
                        # fired its 8 entries (ring is FIFO per queue)
                        prep.wait_op(q0free, 1, "sem-ge")

            # ---- A2A exchange buffers + preps (uniform across cores: the
            # host pre-permutes each core's batch groups so slot k always
            # carries the receiver's own images, and every sender writes its
            # full 4-row window — overlapping rows carry identical values.
            # Only the receive-row offset differs per core: a register offset
            # in the out_ap (symbolic AP), loaded from the myrow input.) ----
            a2a_rsem = nc.alloc_semaphore("a2a_rsem")
            q0free = nc.alloc_semaphore("q0free")
            if USE_RDMA_A2A:
                A2S = cpool.tile([128, 8, 4, 28], bf, tag="A2S", name="A2S")
                base = PBQ2[:, 0:4, :]
                out_sym = (bass.AP(base.tensor, base.offset + rowval, base.ap)
                           if os.environ.get("KERNEL_SYMOFF", "1") == "1"
                           else base)
                for k in range(8):
                    rd = [None] * 8
                    rd[k] = (0, k)
                    nc.gpsimd.remote_dma_broadcast(
                        out_sym, A2S[:, k, :, :],
                        remote_sem=a2a_rsem, local_sem=lsem,
                        rdests=rd, queue_num=0)

            for li in (1, 2, 3):
                emit_stat_preps(li)
            if not USE_RDMA_A2A:
                emit_stat_preps(4)

            def allreduce_stats(li):
                q = QMAP[li]
                """Cross-core sum of SND[li] -> TT [128, F].

                rdma path: trigger queue q's 8 slot-broadcasts; the trigger
                declares SND/RCV as writable signals so tile orders it after
                every SND writer (WAW; remote_dma preps carry no tile-visible
                data deps) and before the RCV slot-sum (RAW). The slot-sum
                additionally gates on the remote sem (register-valued target).
                Fallback: collective AllGather via DRAM bounce buffers."""
                F = FDIM[li]
                TT = spool.tile([128, F], f32, tag=f"TT{li}", name=f"TT{li}")
                if (LOCAL_STATS >> (li - 1)) & 1:
                    nc.vector.tensor_copy(TT[:], SND[li][:])
                elif (RDMA_MASK >> (li - 1)) & 1:
                    nc.gpsimd.trigger_dma(count=None, queue_num=q,
                                          signals_writable=[SND[li][:],
                                                            RCV[li][:]])
                    red = nc.vector.tensor_reduce(
                        TT[:], RCV[li][:].rearrange("p k f -> p f k"),
                        mybir.AxisListType.X, ALU.add)
                    red.wait_op(RSEM[li], wreg, "sem-ge")
                else:
                    sti = dpool.tile([128, F], f32, tag=f"sti{li}",
                                     name=f"sti{li}")
                    sto = dpool.tile([8, 128, F], f32, tag=f"sto{li}",
                                     name=f"sto{li}", addr_space="Shared")
                    nc.gpsimd.dma_start(sti[:], SND[li][:])
                    nc.gpsimd.collective_compute(
                        "AllGather", ALU.bypass, replica_groups=[CORES],
                        ins=[sti.opt()], outs=[sto.opt()])
                    SG = spool.tile([128, 8, F], f32, tag=f"SG{li}",
                                    name=f"SG{li}")
                    nc.gpsimd.dma_start(SG[:], sto[:].rearrange("i c f -> c i f"))
                    nc.vector.tensor_reduce(
                        TT[:], SG[:].rearrange("p k f -> p f k"),
                        mybir.AxisListType.X, ALU.add)
                return TT

            # ================= stage A: locally-connected layers =============
            XP = cpool.tile([28, 8, 32, 32], bf, tag="XP")
            nc.sync.dma_start(XP[:], xp_d[:])

            # patch buffers: partitions (kx*32+c) plus ones-row at 96
            P1 = apool.tile([97, 8, 32, 32], bf, tag="P1")   # LC1 out patches
            P2 = apool.tile([97, 6, 32, 30], bf, tag="P2")   # LC2 out patches
            nc.vector.memset(P2[0:32, :, :, 28:30], 0.0)     # x-pad cols only
            nc.vector.memset(P1[96:97, :, :, :], 1.0)
            nc.vector.memset(P2[96:97, :, :, :], 1.0)
            # LC3 output, laid out for the AllToAll: [o, j(dest core), b, y, x]
            ACT3 = apool.tile([32, 8, 4, 4, 28], bf, tag="ACT3")

            # ---- LC1: out rows 0..8 (local), 32 x-locs (30 true + 2 pad) ----
            for yb in range(4):
                W1t = wpool.tile([28, 2, 8, 128], bf, tag="wrow")
                nc.sync.dma_start(W1t[:], w1p_d[:, 2 * yb:2 * yb + 2])
                PS = pspool.tile([128, 2, 8, 32], f32, tag="ps")
                for gi in range(16):
                    y, g = 2 * yb + gi // 8, gi % 8
                    for li in range(4):
                        nc.tensor.matmul(
                            PS[32 * li:32 * li + 32, gi // 8, g, :],
                            W1t[:, gi // 8, g, 32 * li:32 * li + 32],
                            XP[:, y, :, 4 * g + li],
                            start=True, stop=True,
                            tile_position=(0, 32 * li),
                        )
                for g2 in range(4):
                    nc.scalar.activation(
                        P1[0:32, 2 * yb:2 * yb + 2, :, g2::4]
                          .rearrange("p y b x -> p y x b"),
                        PS[32 * g2:32 * g2 + 32, :, :, :],
                        AF.Prelu, alpha=ALPHA,
                    )
                # x-shifted replicas for partition blocks 1, 2
                nc.vector.tensor_copy(
                    P1[32:64, 2 * yb:2 * yb + 2, :, 0:31],
                    P1[0:32, 2 * yb:2 * yb + 2, :, 1:32])
                nc.vector.tensor_copy(
                    P1[64:96, 2 * yb:2 * yb + 2, :, 0:30],
                    P1[0:32, 2 * yb:2 * yb + 2, :, 2:32])

            # ---- LC2: 6 local rows, 28 x-locs (7 groups exactly) ----
            for y in range(6):
                W2t = wpool.tile([97, 7, 3, 128], bf, tag="wrow")
                nc.sync.dma_start(W2t[:], w2p_d[y])
                PS = pspool.tile([128, 7, 32], f32, tag="ps")
                for g in range(7):
                    for ky in range(3):
                        for li in range(4):
                            nc.tensor.matmul(
                                PS[32 * li:32 * li + 32, g, :],
                                W2t[:, g, ky, 32 * li:32 * li + 32],
                                P1[:, y + ky, :, 4 * g + li],
                                start=(ky == 0), stop=(ky == 2),
                                tile_position=(0, 32 * li),
                            )
                for g2 in range(4):
                    nc.scalar.activation(
                        P2[0:32, y, :, g2:g2 + 25:4].rearrange("p b x -> p x b"),
                        PS[32 * g2:32 * g2 + 32, :, :],
                        AF.Prelu, alpha=ALPHA,
                    )
                nc.vector.tensor_copy(P2[32:64, y, :, 0:28], P2[0:32, y, :, 1:29])
                nc.vector.tensor_copy(P2[64:96, y, :, 0:28], P2[0:32, y, :, 2:30])

            # ---- LC3: 4 local rows, 28 x-locs (26 true + 2 zero-padded) ----
            for y in range(4):
                W3t = wpool.tile([97, 7, 3, 128], bf, tag="wrow")
                nc.sync.dma_start(W3t[:], w3p_d[y])
                PS = pspool.tile([128, 7, 32], f32, tag="ps")
                for g in range(7):
                    for ky in range(3):
                        for li in range(4):
                            nc.tensor.matmul(
                                PS[32 * li:32 * li + 32, g, :],
                                W3t[:, g, ky, 32 * li:32 * li + 32],
                                P2[:, y + ky, :, 4 * g + li],
                                start=(ky == 0), stop=(ky == 2),
                                tile_position=(0, 32 * li),
                            )
                for g2 in range(4):
                    nc.scalar.activation(
                        ACT3[0:32, :, :, y, g2::4].rearrange("p j b x -> p x j b"),
                        PS[32 * g2:32 * g2 + 32, :, :],
                        AF.Prelu, alpha=ALPHA,
                    )

            # ============== transition: AllToAll to batch sharding ===========
            # conv1 input patches: [kx*32+c, b, ypad28, xpad28]; zero only the
            # 1px border of block 0, interior is overwritten
            PB1 = apool.tile([96, 4, 28, 28], bf, tag="P1")
            nc.vector.memset(PB1[0:32, :, 0:1, :], 0.0)
            nc.vector.memset(PB1[0:32, :, 27:28, :], 0.0)
            nc.vector.memset(PB1[0:32, :, 1:27, 0:1], 0.0)
            nc.vector.memset(PB1[0:32, :, 1:27, 27:28], 0.0)
            if USE_RDMA_A2A:
                # repack to 128 partitions (32*b + och); descriptors were
                # prepped above, data is read when the trigger fires
                for b in range(4):
                    nc.sync.dma_start(A2S[32 * b:32 * b + 32, :, :, :],
                                      ACT3[:, :, b, :, :])
                nc.gpsimd.trigger_dma(count=None, queue_num=0,
                                      signals_writable=[A2S[:], PBQ2[:]])
                for b in range(4):
                    cp = nc.vector.tensor_copy(
                        PB1[0:32, b, 1:27, 1:27],
                        PBQ2[32 * b:32 * b + 32, :, 0:26])
                    cp.wait_op(a2a_rsem, wreg, "sem-ge")
                rel = nc.vector.sem_inc(q0free, 1)
                rel.wait_op(a2a_rsem, wreg, "sem-ge")
            else:
                a2a_in = dpool.tile([8, 32, 4, 4, 28], bf, tag="a2a_in")
                a2a_out = dpool.tile([8, 32, 4, 4, 28], bf, tag="a2a_out")
                nc.gpsimd.dma_start(
                    a2a_in[:].rearrange("j o b y x -> o j (b y x)"),
                    ACT3[:].rearrange("p j b y x -> p j (b y x)"))
                nc.gpsimd.collective_compute(
                    "AllToAll", ALU.bypass, replica_groups=[CORES],
                    ins=[a2a_in.opt()], outs=[a2a_out.opt()])
                # one bulk DMA for the whole A2A result, then cheap DVE
                # row-selection copies (8 small DMAs each pay ~1.5us of DGE
                # + semaphore latency, serializing ~15us after the A2A)
                PBA = apool.tile([32, 8, 4, 4, 28], bf, tag="ACT3")
                nc.gpsimd.dma_start(
                    PBA[:], a2a_out[:].rearrange("i o b y x -> o i b y x"))
                for i in range(NCORES):
                    lo, hi = TAKE[i]
                    nc.vector.tensor_copy(
                        PB1[0:32, :, 1 + lo:1 + hi, 1:27],
                        PBA[:, i, :, lo - ST[i]:hi - ST[i], 0:26])
            nc.vector.tensor_copy(PB1[32:64, :, :, 0:27], PB1[0:32, :, :, 1:28])
            nc.vector.tensor_copy(PB1[64:96, :, :, 0:26], PB1[0:32, :, :, 2:28])
            if USE_RDMA_A2A:
                emit_stat_preps(4)

            # eps tile for sqrt(var + eps)
            EPST = spool.tile([128, 1], f32, tag="EPST")
            nc.vector.memset(EPST[:], EPS)

            def bn_scale_shift(tag, C, TTs, TTq, bn_g, bn_b, n_elems):
                """scale/shift from total sum TTs / sumsq TTq ([C,1] views)."""
                MEAN = spool.tile([C, 1], f32, tag=f"MEAN{tag}")
                MSQ = spool.tile([C, 1], f32, tag=f"MSQ{tag}")
                VAR = spool.tile([C, 1], f32, tag=f"VAR{tag}")
                SD = spool.tile([C, 1], f32, tag=f"SD{tag}")
                SC = spool.tile([C, 1], f32, tag=f"SC{tag}")
                TB = spool.tile([C, 1], f32, tag=f"TB{tag}")
                nc.scalar.mul(MEAN[:], TTs, 1.0 / n_elems)
                nc.scalar.mul(MSQ[:], TTq, 1.0 / n_elems)
                nc.vector.tensor_mul(VAR[:], MEAN[:], MEAN[:])
                nc.vector.tensor_sub(VAR[:], MSQ[:], VAR[:])
                nc.scalar.activation(SD[:], VAR[:], AF.Sqrt, bias=EPST[0:C, :])
                nc.vector.reciprocal(SD[:], SD[:])
                nc.vector.tensor_mul(SC[:], bn_g, SD[:])
                nc.vector.tensor_mul(TB[:], MEAN[:], SC[:])
                nc.vector.tensor_sub(TB[:], bn_b, TB[:])
                return SC, TB

            # ======================= conv1 + BN + pool =======================
            W1B = cpool.tile([96, 3, 64], bf, tag="W1B")
            nc.sync.dma_start(W1B[:], w1b_d[:])
            BN1 = cpool.tile([64, 2], f32, tag="BN1")
            nc.sync.dma_start(BN1[:], bn1_d[:])
            O1 = apool.tile([64, 4, 2, 13, 26], f32, tag="P2")  # (b, yh, y13, x26)
            SA1 = spool.tile([64, 8], f32, tag="SA1")
            QA1 = spool.tile([64, 8], f32, tag="QA1")
            for nb in range(8):
                b, yh = nb // 2, nb % 2
                PS = pspool.tile([64, 13, 26], f32, tag="ps")
                for ky in range(3):
                    nc.tensor.matmul(
                        PS[:], W1B[:, ky, :],
                        PB1[0:96, b, 13 * yh + ky:13 * yh + ky + 13, 0:26],
                        start=(ky == 0), stop=(ky == 2))
                nc.scalar.activation(O1[:, b, yh, :, :], PS[:], AF.Copy,
                                     accum_out=SA1[:, nb:nb + 1])
                SCR = scrpool.tile([64, 13, 26], f32, tag="scr")
                nc.scalar.activation(SCR[:], O1[:, b, yh, :, :], AF.Square,
                                     accum_out=QA1[:, nb:nb + 1])
            nc.vector.memset(SND[1][64:128, :], 0.0)
            nc.vector.tensor_reduce(SND[1][0:64, 0:1], SA1[:],
                                    mybir.AxisListType.X, ALU.add)
            nc.vector.tensor_reduce(SND[1][0:64, 1:2], QA1[:],
                                    mybir.AxisListType.X, ALU.add)
            TT1 = allreduce_stats(1)
            SC1, TB1 = bn_scale_shift("1", 64, TT1[0:64, 0:1], TT1[0:64, 1:2],
                                      BN1[:, 0:1], BN1[:, 1:2], N1)
            PA = apool.tile([64, 4, 26, 13], f32, tag="PA")
            T1 = O1[:].rearrange("p b h y x -> p b (h y) x")
            for bh in range(2):
                nc.scalar.activation(O1[:, 2 * bh:2 * bh + 2], 
                                     O1[:, 2 * bh:2 * bh + 2], AF.Prelu,
                                     bias=TB1[:], scale=SC1[:], alpha=ALPHA)
                nc.vector.tensor_max(PA[:, 2 * bh:2 * bh + 2],
                                     T1[:, 2 * bh:2 * bh + 2, :, 0::2],
                                     T1[:, 2 * bh:2 * bh + 2, :, 1::2])
            PB2a = apool.tile([128, 4, 15, 15], bf, tag="ACT3")
            PB2b = apool.tile([64, 4, 15, 15], bf, tag="PB2b")
            nc.vector.memset(PB2a[0:64, :, 0:1, :], 0.0)
            nc.vector.memset(PB2a[0:64, :, 14:15, :], 0.0)
            nc.vector.memset(PB2a[0:64, :, 1:14, 0:1], 0.0)
            nc.vector.memset(PB2a[0:64, :, 1:14, 14:15], 0.0)
            nc.vector.tensor_max(PB2a[0:64, :, 1:14, 1:14],
                                 PA[:, :, 0:26:2, :], PA[:, :, 1:26:2, :])
            nc.vector.tensor_copy(PB2a[64:128, :, :, 0:14], PB2a[0:64, :, :, 1:15])
            nc.vector.tensor_copy(PB2b[0:64, :, :, 0:13], PB2a[0:64, :, :, 2:15])

            # ======================= conv2 + BN + pool =======================
            W2BA = cpool.tile([128, 3, 128], bf, tag="W2BA")
            nc.sync.dma_start(W2BA[:], w2ba_d[:])
            W2BB = cpool.tile([64, 3, 128], bf, tag="W2BB")
            nc.sync.dma_start(W2BB[:], w2bb_d[:])
            BN2 = cpool.tile([128, 2], f32, tag="BN2")
            nc.sync.dma_start(BN2[:], bn2_d[:])
            O2 = apool.tile([128, 4, 13, 13], f32, tag="O2")
            SA2 = spool.tile([128, 4], f32, tag="SA2")
            QA2 = spool.tile([128, 4], f32, tag="QA2")
            for b in range(4):
                PS = pspool.tile([128, 13, 13], f32, tag="ps")
                for ky in range(3):
                    nc.tensor.matmul(PS[:], W2BA[:, ky, :],
                                     PB2a[:, b, ky:ky + 13, 0:13],
                                     start=(ky == 0), stop=False)
                for ky in range(3):
                    nc.tensor.matmul(PS[:], W2BB[:, ky, :],
                                     PB2b[:, b, ky:ky + 13, 0:13],
                                     start=False, stop=(ky == 2))
                nc.scalar.activation(O2[:, b, :, :], PS[:], AF.Copy,
                                     accum_out=SA2[:, b:b + 1])
                SCR = scrpool.tile([128, 13, 13], f32, tag="scr")
                nc.scalar.activation(SCR[:], O2[:, b, :, :], AF.Square,
                                     accum_out=QA2[:, b:b + 1])
            nc.vector.tensor_reduce(SND[2][:, 0:1], SA2[:],
                                    mybir.AxisListType.X, ALU.add)
            nc.vector.tensor_reduce(SND[2][:, 1:2], QA2[:],
                                    mybir.AxisListType.X, ALU.add)
            TT2 = allreduce_stats(2)
            SC2, TB2 = bn_scale_shift("2", 128, TT2[:, 0:1], TT2[:, 1:2],
                                      BN2[:, 0:1], BN2[:, 1:2], N2)
            PA2 = apool.tile([128, 4, 12, 6], f32, tag="PA2")
            for bh in range(2):
                nc.scalar.activation(O2[:, 2 * bh:2 * bh + 2],
                                     O2[:, 2 * bh:2 * bh + 2], AF.Prelu,
                                     bias=TB2[:], scale=SC2[:], alpha=ALPHA)
                nc.vector.tensor_max(PA2[:, 2 * bh:2 * bh + 2],
                                     O2[:, 2 * bh:2 * bh + 2, 0:12, 0:12:2],
                                     O2[:, 2 * bh:2 * bh + 2, 0:12, 1:13:2])
            PB3a = apool.tile([128, 4, 8, 8], bf, tag="P1")
            PB3b = apool.tile([128, 4, 8, 8], bf, tag="PB3b")
            PB3c = apool.tile([128, 4, 8, 8], bf, tag="PB3c")
            nc.vector.memset(PB3a[:, :, 0:1, :], 0.0)
            nc.vector.memset(PB3a[:, :, 7:8, :], 0.0)
            nc.vector.memset(PB3a[:, :, 1:7, 0:1], 0.0)
            nc.vector.memset(PB3a[:, :, 1:7, 7:8], 0.0)
            nc.vector.tensor_max(PB3a[:, :, 1:7, 1:7],
                                 PA2[:, :, 0:12:2, :], PA2[:, :, 1:12:2, :])
            nc.vector.tensor_copy(PB3b[:, :, :, 0:7], PB3a[:, :, :, 1:8])
            nc.vector.tensor_copy(PB3c[:, :, :, 0:6], PB3a[:, :, :, 2:8])

            # ======================= conv3 + BN + pool =======================
            W3B = cpool.tile([128, 3, 3, 256], bf, tag="W3B")
            nc.sync.dma_start(W3B[:], w3b_d[:])
            BN3 = cpool.tile([128, 2, 2], f32, tag="BN3")
            nc.sync.dma_start(BN3[:], bn3_d[:])
            O3 = apool.tile([128, 2, 4, 6, 6], f32, tag="O3")  # (mh, b, y, x)
            SA3 = spool.tile([128, 2], f32, tag="SA3")
            QA3 = spool.tile([128, 2], f32, tag="QA3")
            PBs = [PB3a, PB3b, PB3c]
            for mh in range(2):
                PS = pspool.tile([128, 4, 6, 6], f32, tag="ps")
                for ky in range(3):
                    for kx in range(3):
                        nc.tensor.matmul(
                            PS[:], W3B[:, ky, kx, 128 * mh:128 * mh + 128],
                            PBs[kx][:, :, ky:ky + 6, 0:6],
                            start=(ky == 0 and kx == 0), stop=(ky == 2 and kx == 2))
                nc.scalar.activation(O3[:, mh, :, :, :], PS[:], AF.Copy,
                                     accum_out=SA3[:, mh:mh + 1])
                SCR = scrpool.tile([128, 4, 6, 6], f32, tag="scr")
                nc.scalar.activation(SCR[:], O3[:, mh, :, :, :], AF.Square,
                                     accum_out=QA3[:, mh:mh + 1])
            nc.vector.tensor_copy(SND[3][:, 0:4:2], SA3[:])
            nc.vector.tensor_copy(SND[3][:, 1:4:2], QA3[:])
            TT3 = allreduce_stats(3)
            SC3, TB3 = {}, {}
            for mh in range(2):
                SC3[mh], TB3[mh] = bn_scale_shift(
                    f"3{mh}", 128, TT3[:, 2 * mh:2 * mh + 1],
                    TT3[:, 2 * mh + 1:2 * mh + 2],
                    BN3[:, mh, 0:1], BN3[:, mh, 1:2], N3)
            PB4 = []
            for kx in range(3):
                row = []
                for mh in range(2):
                    pb4t = apool.tile([128, 4, 5, 5], bf, tag=f"PB4{kx}{mh}")
                    row.append(pb4t)
                PB4.append(row)
            for mh in range(2):
                nc.scalar.activation(O3[:, mh, :, :, :], O3[:, mh, :, :, :],
                                     AF.Prelu, bias=TB3[mh][:], scale=SC3[mh][:],
                                     alpha=ALPHA)
                T3 = O3[:, mh, :, :, :]
                PA3 = apool.tile([128, 4, 6, 3], f32, tag=f"PA3{mh}")
                nc.vector.tensor_max(PA3[:], T3[:, :, :, 0::2], T3[:, :, :, 1::2])
                nc.vector.memset(PB4[0][mh][:, :, 0:1, :], 0.0)
                nc.vector.memset(PB4[0][mh][:, :, 4:5, :], 0.0)
                nc.vector.memset(PB4[0][mh][:, :, 1:4, 0:1], 0.0)
                nc.vector.memset(PB4[0][mh][:, :, 1:4, 4:5], 0.0)
                nc.vector.tensor_max(PB4[0][mh][:, :, 1:4, 1:4],
                                     PA3[:, :, 0:6:2, :], PA3[:, :, 1:6:2, :])
                nc.vector.tensor_copy(PB4[1][mh][:, :, :, 0:4],
                                      PB4[0][mh][:, :, :, 1:5])
                nc.vector.tensor_copy(PB4[2][mh][:, :, :, 0:3],
                                      PB4[0][mh][:, :, :, 2:5])

            # ======================= convf + BN + tanh =======================
            WFB = cpool.tile([128, 2, 3, 3, 256], bf, tag="WFB")
            nc.sync.dma_start(WFB[:], wfb_d[:])
            BNF = cpool.tile([128, 2, 2], f32, tag="BNF")
            nc.sync.dma_start(BNF[:], bnf_d[:])
            OF = apool.tile([128, 2, 4, 3, 3], f32, tag="OF")
            SAF = spool.tile([128, 2], f32, tag="SAF")
            QAF = spool.tile([128, 2], f32, tag="QAF")
            for mh in range(2):
                PS = pspool.tile([128, 4, 3, 3], f32, tag="ps")
                first = True
                for cb in range(2):
                    for ky in range(3):
                        for kx in range(3):
                            nc.tensor.matmul(
                                PS[:], WFB[:, cb, ky, kx, 128 * mh:128 * mh + 128],
                                PB4[kx][cb][:, :, ky:ky + 3, 0:3],
                                start=first, stop=(cb == 1 and ky == 2 and kx == 2))
                            first = False
                nc.scalar.activation(OF[:, mh, :, :, :], PS[:], AF.Copy,
                                     accum_out=SAF[:, mh:mh + 1])
                SCR = scrpool.tile([128, 4, 3, 3], f32, tag="scr")
                nc.scalar.activation(SCR[:], OF[:, mh, :, :, :], AF.Square,
                                     accum_out=QAF[:, mh:mh + 1])
            nc.vector.tensor_copy(SND[4][:, 0:4:2], SAF[:])
            nc.vector.tensor_copy(SND[4][:, 1:4:2], QAF[:])
            TTF = allreduce_stats(4)
            for mh in range(2):
                SCt, TBt = bn_scale_shift(
                    f"f{mh}", 128, TTF[:, 2 * mh:2 * mh + 1],
                    TTF[:, 2 * mh + 1:2 * mh + 2],
                    BNF[:, mh, 0:1], BNF[:, mh, 1:2], NF)
                OUTT = apool.tile([128, 4, 3, 3], f32, tag=f"OUTT{mh}")
                nc.scalar.activation(OUTT[:], OF[:, mh, :, :, :], AF.Tanh,
                                     bias=TBt[:], scale=SCt[:])
                nc.sync.dma_start(
                    out_d[:, 128 * mh:128 * mh + 128, :, :]
                        .rearrange("b c y x -> c b y x"),
                    OUTT[:])
    return nc


def _prep(inputs):
    """Host-side shard + layout prep. Pure data movement (plus dtype cast)."""
    f32 = np.float32
    x = np.asarray(inputs["x"], f32)
    lc1_w = np.asarray(inputs["lc1_w"], f32)[0]  # (32,3,30,30,9)
    lc1_b = np.asarray(inputs["lc1_b"], f32)[0]  # (32,30,30)
    lc2_w = np.asarray(inputs["lc2_w"], f32)[0]  # (32,32,28,28,9)
    lc2_b = np.asarray(inputs["lc2_b"], f32)[0]
    lc3_w = np.asarray(inputs["lc3_w"], f32)[0]  # (32,32,26,26,9)
    lc3_b = np.asarray(inputs["lc3_b"], f32)[0]

    # replicated stage-B weights
    c1w = np.asarray(inputs["c1_w"], f32)
    c2w = np.asarray(inputs["c2_w"], f32)
    c3w = np.asarray(inputs["c3_w"], f32)
    cfw = np.asarray(inputs["cf_w"], f32)
    w1b = np.ascontiguousarray(c1w.transpose(3, 1, 2, 0).reshape(96, 3, 64)).astype(CAST)
    w2ba = np.ascontiguousarray(
        c2w[:, :, :, 0:2].transpose(3, 1, 2, 0).reshape(128, 3, 128)).astype(CAST)
    w2bb = np.ascontiguousarray(c2w[:, :, :, 2].transpose(1, 2, 0)).astype(CAST)
    w3b = np.ascontiguousarray(c3w.transpose(1, 2, 3, 0)).astype(CAST)
    wfb = np.ascontiguousarray(
        cfw.reshape(256, 2, 128, 3, 3).transpose(2, 1, 3, 4, 0)).astype(CAST)
    bn1 = np.stack([np.asarray(inputs["c1_g"], f32),
                    np.asarray(inputs["c1_beta"], f32)], axis=1)
    bn2 = np.stack([np.asarray(inputs["c2_g"], f32),
                    np.asarray(inputs["c2_beta"], f32)], axis=1)
    bn3 = np.stack([np.asarray(inputs["c3_g"], f32).reshape(2, 128).T,
                    np.asarray(inputs["c3_beta"], f32).reshape(2, 128).T], axis=2)
    bnf = np.stack([np.asarray(inputs["cf_g"], f32).reshape(2, 128).T,
                    np.asarray(inputs["cf_beta"], f32).reshape(2, 128).T], axis=2)

    def lc_pack(wsl, bsl, nrow, width):
        """wsl: (32o,32c,nrow,width,9) -> (nrow, 97, G, 3, 128); bsl: (32o,nrow,width)"""
        G = 7
        wp = np.zeros((32, 32, nrow, 4 * G, 9), f32)
        wp[:, :, :, :width] = wsl
        bp = np.zeros((32, nrow, 4 * G), f32)
        bp[:, :, :width] = bsl
        arr = wp.reshape(32, 32, nrow, G, 4, 3, 3)  # o c y g li ky kx
        arr = arr.transpose(2, 6, 1, 3, 5, 4, 0).reshape(nrow, 96, G, 3, 128)
        outw = np.zeros((nrow, 97, G, 3, 128), f32)
        outw[:, :96] = arr
        outw[:, 96, :, 0, :] = bp.transpose(1, 2, 0).reshape(nrow, G, 4, 32)\
                                 .reshape(nrow, G, 128)
        return outw.astype(CAST)

    # core c computes LC3 rows ST[lab[c]] and stage-B images [4*lab[c]..);
    # with the rdma A2A, lab is the XOR label from discovery, else identity
    lab = _discover_dmap() if USE_RDMA_A2A else list(range(NCORES))

    in_maps = []
    xpad = np.zeros((32, 3, 32, 34), f32)
    xpad[:, :, :, :32] = x
    for c in range(NCORES):
        s = ST[lab[c]]
        if USE_RDMA_A2A:
            # batch-group permutation: XP group m holds image group lab^m,
            # so A2A slot m always carries the receiver's own images
            border = np.concatenate(
                [np.arange(4 * (lab[c] ^ m), 4 * (lab[c] ^ m) + 4)
                 for m in range(8)])
        else:
            border = np.arange(32)
        xb = xpad[border]
        xp = np.zeros((28, 8, 32, 32), f32)
        for ky in range(3):
            for kx in range(3):
                k = ky * 3 + kx
                blk = xb[:, :, s + ky:s + ky + 8, kx:kx + 32]  # (b,c,y,x)
                xp[3 * k:3 * k + 3] = blk.transpose(1, 2, 0, 3)
        xp[27] = 1.0

        w1sl = np.zeros((32, 3, 8, 32, 9), f32)
        w1sl[:, :, :, :30] = lc1_w[:, :, s:s + 8]
        b1sl = np.zeros((32, 8, 32), f32)
        b1sl[:, :, :30] = lc1_b[:, s:s + 8]
        arr = w1sl.reshape(32, 3, 8, 8, 4, 9)  # o c y g li k
        arr = arr.transpose(5, 1, 2, 3, 4, 0).reshape(27, 8, 8, 128)
        w1p = np.zeros((28, 8, 8, 128), f32)
        w1p[:27] = arr
        w1p[27] = b1sl.transpose(1, 2, 0).reshape(8, 8, 4, 32).reshape(8, 8, 128)

        w2p = lc_pack(lc2_w[:, :, s:s + 6], lc2_b[:, s:s + 6], 6, 28)
        w3p = lc_pack(lc3_w[:, :, s:s + 4], lc3_b[:, s:s + 4], 4, 26)

        in_maps.append({
            "xp": xp.astype(CAST), "w1p": w1p.astype(CAST),
            "w2p": w2p, "w3p": w3p,
            "w1b": w1b, "w2ba": w2ba, "w2bb": w2bb, "w3b": w3b, "wfb": wfb,
            "bn1": bn1, "bn2": bn2, "bn3": bn3, "bnf": bnf,
            "myrow": np.array([[ST[lab[c]] * 28]], np.uint32),
        })
    return in_maps


def get_nc():
    if "nc" not in _cache:
        nc = _build()
        nc.compile()
        _cache["nc"] = nc
    return _cache["nc"]


def kernel(**inputs) -> np.ndarray:
    nc = get_nc()
    in_maps = _prep(inputs)
    lab = _discover_dmap() if USE_RDMA_A2A else list(range(NCORES))
    res = run_bass_kernel_spmd(nc, in_maps, CORES)
    out = np.empty((32, 256, 3, 3), np.float32)
    for c in range(NCORES):
        out[4 * lab[c]:4 * lab[c] + 4] = res.results[c]["out"]
    return out



# revision 5
# speedup vs baseline: 21.4573x; 21.4573x over previous
"""Trainium2 Bass kernel for nn_Locally_Connected_Module.

Network: 3 locally-connected 3x3 layers (per-location weights, ~57MB total),
then 4 conv3x3+BN(+PReLU/tanh) blocks with 3 maxpools.
  x (32,3,32,32) -> LC1 -> (32,32,30,30) -> LC2 -> (32,32,28,28) -> LC3 ->
  (32,32,26,26) -> conv1+bn+prelu+pool -> (32,64,13,13) -> conv2.. ->
  (32,128,6,6) -> conv3.. -> (32,256,3,3) -> convf+bn+tanh -> (32,256,3,3)

Sharding:
  Stage A (LC layers): SPATIAL row-sharding over the 8 cores; each core
  computes a 4-row slice of LC3 output (with halo back through LC2/LC1) for
  the FULL batch, so each core only reads ~1/8 of the per-location LC
  weights. LC bias is folded in as a K=97th "ones" row.
  Transition: AllToAll converts (all batch, row slice) -> (4 images,
  all rows) per core.
  Stage B (convs): batch-parallel, 4 images/core. Train-mode BN batch stats
  are reduced cross-core with small AllGathers (partial sum/sumsq per core,
  summed locally after the gather). Conv biases are skipped: train-mode BN
  makes them no-ops. Final output is batch-sharded; host concatenates.

Compute dtype: fp16 operands (PE matmul is 4x faster than fp32 and fp16
keeps ~11 mantissa bits vs bf16's 8, landing at ~7e-3 max rel err) with
fp32 PSUM accumulation and fp32 BN statistics math. Memsets only touch pad
borders; patch-shift replicas are width-trimmed to the consumed columns.

Experimental (env-gated OFF by default): KERNEL_RDMA/KERNEL_A2A switch the
BN-stat reductions and the AllToAll to XOR-slot remote_dma_broadcast
exchanges (~2us vs the ~15us fixed cost of each collective). The stats
exchange and the A2A each verify standalone (see probe_rdma*.py: XOR-slot
routing, symbolic row-offset out_ap, register-valued remote-sem waits),
but the combined full kernel still hits an opaque device fault (ruled
out: tile-pool reuse racing remote writes -- receive buffers are raw SBUF
tensors now; SWDGE ring overflow -- each broadcast prep is 66 descriptors
vs a 1024/queue ring, and raising dynamic_dma_scratch_size did not help).
collective_compute + remote_dma also cannot coexist in one NEFF (hangs),
so the verified collective path ships.
"""
import numpy as np

import concourse.bass as bass
import concourse.bacc as bacc
import concourse.mybir as mybir
import concourse.tile as tile
from concourse.bass_utils import run_bass_kernel_spmd
from concourse import bass2jax

dt = mybir.dt
AF = mybir.ActivationFunctionType
ALU = mybir.AluOpType

import os
USE_FP16 = os.environ.get("KERNEL_FP16", "1") == "1"
# bitmask: bit (li-1) set -> BN layer li uses the remote_dma stat exchange;
# clear -> collective AllGather fallback
RDMA_MASK = int(os.environ.get("KERNEL_RDMA", "0"))
# AllToAll via remote_dma (Switch on the core's XOR label) vs collective.
# NOTE: collective_compute and remote_dma hang when mixed in one NEFF, so
# A2A=1 requires RDMA_MASK=15 and A2A=0 requires RDMA_MASK=0.
USE_RDMA_A2A = os.environ.get("KERNEL_A2A", "0") == "1"
# debug bitmask: bit (li-1) -> use local-only stats for BN layer li
LOCAL_STATS = int(os.environ.get("KERNEL_LOCALSTATS", "0"))
CAST = np.float16 if USE_FP16 else np.float32

NCORES = 8
CORES = list(range(NCORES))
EPS = 1e-5
ALPHA = 0.25

# LC3 output row starts per core (each computes rows [s, s+4) of 26)
ST = [0, 4, 8, 11, 14, 17, 20, 22]
# which global rows to take from each core's chunk when reassembling
TAKE = [(0, 4), (4, 8), (8, 12), (12, 15), (15, 18), (18, 21), (21, 24), (24, 26)]

N1, N2, N3, NF = 32 * 26 * 26, 32 * 13 * 13, 32 * 6 * 6, 32 * 3 * 3

_cache = {}


def _build_discovery():
    """Tiny NEFF: each core XOR-slot-broadcasts its logical id; receivers'
    slot k then holds the logical id of the core whose physical NC id is
    (own_pid ^ k). Core 0's slots give each logical core's XOR label d:
    d[slots0[k]] = k. Needed because the driver's logical->physical NC map
    is not readable from the client."""
    nc = bacc.Bacc("TRN2", target_bir_lowering=False)
    f32 = dt.float32
    val_d = nc.dram_tensor("val", [128, 1], f32, kind="ExternalInput")
    slots_d = nc.dram_tensor("slots", [128, 8], f32, kind="ExternalOutput")
    wreg = nc.vector.alloc_register("rwait")
    nc.vector.reg_mov(wreg, 16)
    with tile.TileContext(nc) as tc:
        with tc.tile_pool(name="p", bufs=1) as pool:
            VAL = pool.tile([128, 1], f32, tag="VAL")
            nc.sync.dma_start(VAL[:], val_d[:])
            RECV = pool.tile([128, 8], f32, tag="RECV")
            rsem = nc.alloc_semaphore("rsem")
            lsem = nc.alloc_semaphore("lsem")
            for k in range(8):
                rd = [None] * 8
                rd[k] = (0, k)
                nc.gpsimd.remote_dma_broadcast(
                    RECV[:, k:k + 1], VAL[:],
                    remote_sem=rsem, local_sem=lsem, rdests=rd)
            nc.gpsimd.trigger_dma(count=None)
            SL = pool.tile([128, 8], f32, tag="SL")
            cp = nc.vector.tensor_copy(SL[:], RECV[:])
            cp.wait_op(rsem, wreg, "sem-ge")
            nc.sync.dma_start(slots_d[:], SL[:])
    return nc


def _discover_dmap():
    """Run the discovery NEFF once; return d[logical] = XOR label."""
    if "dmap" not in _cache:
        nc = _build_discovery()
        nc.compile()
        in_maps = [{"val": np.full((128, 1), float(c), np.float32)}
                   for c in range(NCORES)]
        res = run_bass_kernel_spmd(nc, in_maps, CORES)
        slots0 = res.results[0]["slots"][0]  # [8] sender logical ids
        d = [-1] * NCORES
        for k in range(NCORES):
            d[int(round(float(slots0[k])))] = k
        assert sorted(d) == list(range(NCORES)), f"bad discovery: {slots0}"
        _cache["dmap"] = d
    return _cache["dmap"]


def _build():
    nc = bacc.Bacc("TRN2", target_bir_lowering=False, num_swdge_queues=4)
    f32 = dt.float32
    bf = dt.float16 if USE_FP16 else dt.float32

    # ---- external inputs (per-core data, same shapes on all cores) ----
    xp_d = nc.dram_tensor("xp", [28, 8, 32, 32], bf, kind="ExternalInput")
    w1p_d = nc.dram_tensor("w1p", [28, 8, 8, 128], bf, kind="ExternalInput")
    w2p_d = nc.dram_tensor("w2p", [6, 97, 7, 3, 128], bf, kind="ExternalInput")
    w3p_d = nc.dram_tensor("w3p", [4, 97, 7, 3, 128], bf, kind="ExternalInput")
    w1b_d = nc.dram_tensor("w1b", [96, 3, 64], bf, kind="ExternalInput")
    w2ba_d = nc.dram_tensor("w2ba", [128, 3, 128], bf, kind="ExternalInput")
    w2bb_d = nc.dram_tensor("w2bb", [64, 3, 128], bf, kind="ExternalInput")
    w3b_d = nc.dram_tensor("w3b", [128, 3, 3, 256], bf, kind="ExternalInput")
    wfb_d = nc.dram_tensor("wfb", [128, 2, 3, 3, 256], bf, kind="ExternalInput")
    bn1_d = nc.dram_tensor("bn1", [64, 2], f32, kind="ExternalInput")
    bn2_d = nc.dram_tensor("bn2", [128, 2], f32, kind="ExternalInput")
    bn3_d = nc.dram_tensor("bn3", [128, 2, 2], f32, kind="ExternalInput")
    bnf_d = nc.dram_tensor("bnf", [128, 2, 2], f32, kind="ExternalInput")

    out_d = nc.dram_tensor("out", [4, 256, 3, 3], bf, kind="ExternalOutput")
    # per-core row offset ST[label]*28 for the A2A receive window
    myrow_d = nc.dram_tensor("myrow", [1, 1], dt.uint32, kind="ExternalInput")

    # register-valued wait target for remote-sem gates (preamble block so it
    # is set before any tile-scheduled instruction decodes a wait against it)
    wreg = nc.vector.alloc_register("rdma_wait16")
    nc.vector.reg_mov(wreg, 16)
    rowreg = nc.gpsimd.alloc_register("myrow_reg")
    nc.gpsimd.reg_load(rowreg, myrow_d[0:1, 0:1])
    rowval = nc.gpsimd.snap(rowreg, donate=True, min_val=0, max_val=22 * 28)

    # remote-write targets live OUTSIDE the tile pools: peers' RDMA writes
    # land asynchronously and must never race tile-pool buffer reuse
    FDIM = {1: 2, 2: 2, 3: 4, 4: 4}  # free f32 elems per layer tag
    RCV = {li: nc.alloc_sbuf_tensor(f"RCV{li}", [128, 8, FDIM[li]], f32)
           for li in (1, 2, 3, 4)}
    PBQ2 = nc.alloc_sbuf_tensor("PBQ2", [128, 26, 28], bf)

    with tile.TileContext(nc) as tc:
        with (
            tc.tile_pool(name="const", bufs=1) as cpool,
            tc.tile_pool(name="wrow", bufs=3) as wpool,
            tc.tile_pool(name="act", bufs=1) as apool,
            tc.tile_pool(name="stat", bufs=1) as spool,
            tc.tile_pool(name="scr", bufs=2) as scrpool,
            tc.tile_pool(name="psum", bufs=4, space="PSUM") as pspool,
            tc.tile_pool(name="dram", bufs=1, space="DRAM") as dpool,
        ):
            # ---- BN-stat exchange buffers + preps (hoisted; data deps are
            # deferred to each queue's trigger) ----
            # queue map: A2A shares q0 with BNf (BNf preps are emitted after
            # the A2A trigger, so the q0 ring order stays A2A -> BNf)
            QMAP = {1: 1, 2: 2, 3: 3, 4: 0} if USE_RDMA_A2A \
                else {1: 0, 2: 1, 3: 2, 4: 3}
            SND, RSEM = {}, {}
            lsem = nc.alloc_semaphore("rdma_lsem")
            for li in (1, 2, 3, 4):
                SND[li] = spool.tile([128, FDIM[li]], f32, tag=f"SND{li}",
                                     name=f"SND{li}")
                RSEM[li] = nc.alloc_semaphore(f"rsem{li}")

            def emit_stat_preps(li):
                if (LOCAL_STATS >> (li - 1)) & 1 or not (RDMA_MASK >> (li - 1)) & 1:
                    return
                for k in range(8):
                    rd = [None] * 8
                    rd[k] = (0, k)
                    prep = nc.gpsimd.remote_dma_broadcast(
                        RCV[li][:, k, :], SND[li][:],
                        remote_sem=RSEM[li], local_sem=lsem,
                        rdests=rd, queue_num=QMAP[li])
                    if USE_RDMA_A2A and QMAP[li] == 0:
                        # q0 is shared with the A2A: descriptor generation
                        # must not enter the ring before the A2A trigger has
                        # fired its 8 entries (ring is FIFO per queue)
                        prep.wait_op(q0free, 1, "sem-ge")

            # ---- A2A exchange buffers + preps (uniform across cores: the
            # host pre-permutes each core's batch groups so slot k always
            # carries the receiver's own images, and every sender writes its
            # full 4-row window — overlapping rows carry identical values.
            # Only the receive-row offset differs per core: a register offset
            # in the out_ap (symbolic AP), loaded from the myrow input.) ----
            a2a_rsem = nc.alloc_semaphore("a2a_rsem")
            q0free = nc.alloc_semaphore("q0free")
            if USE_RDMA_A2A:
                A2S = cpool.tile([128, 8, 4, 28], bf, tag="A2S", name="A2S")
                base = PBQ2[:, 0:4, :]
                out_sym = (bass.AP(base.tensor, base.offset + rowval, base.ap)
                           if os.environ.get("KERNEL_SYMOFF", "1") == "1"
                           else base)
                for k in range(8):
                    rd = [None] * 8
                    rd[k] = (0, k)
                    nc.gpsimd.remote_dma_broadcast(
                        out_sym, A2S[:, k, :, :],
                        remote_sem=a2a_rsem, local_sem=lsem,
                        rdests=rd, queue_num=0)

            for li in (1, 2, 3):
                emit_stat_preps(li)
            if not USE_RDMA_A2A:
                emit_stat_preps(4)

            def allreduce_stats(li):
                q = QMAP[li]
                """Cross-core sum of SND[li] -> TT [128, F].

                rdma path: trigger queue q's 8 slot-broadcasts; the trigger
                declares SND/RCV as writable signals so tile orders it after
                every SND writer (WAW; remote_dma preps carry no tile-visible
                data deps) and before the RCV slot-sum (RAW). The slot-sum
                additionally gates on the remote sem (register-valued target).
                Fallback: collective AllGather via DRAM bounce buffers."""
                F = FDIM[li]
                TT = spool.tile([128, F], f32, tag=f"TT{li}", name=f"TT{li}")
                if (LOCAL_STATS >> (li - 1)) & 1:
                    nc.vector.tensor_copy(TT[:], SND[li][:])
                elif (RDMA_MASK >> (li - 1)) & 1:
                    nc.gpsimd.trigger_dma(count=None, queue_num=q,
                                          signals_writable=[SND[li][:],
                                                            RCV[li][:]])
                    red = nc.vector.tensor_reduce(
                        TT[:], RCV[li][:].rearrange("p k f -> p f k"),
                        mybir.AxisListType.X, ALU.add)
                    red.wait_op(RSEM[li], wreg, "sem-ge")
                else:
                    sti = dpool.tile([128, F], f32, tag=f"sti{li}",
                                     name=f"sti{li}")
                    sto = dpool.tile([8, 128, F], f32, tag=f"sto{li}",
                                     name=f"sto{li}", addr_space="Shared")
                    nc.gpsimd.dma_start(sti[:], SND[li][:])
                    nc.gpsimd.collective_compute(
                        "AllGather", ALU.bypass, replica_groups=[CORES],
                        ins=[sti.opt()], outs=[sto.opt()])
                    SG = spool.tile([128, 8, F], f32, tag=f"SG{li}",
                                    name=f"SG{li}")
                    nc.gpsimd.dma_start(SG[:], sto[:].rearrange("i c f -> c i f"))
                    nc.vector.tensor_reduce(
                        TT[:], SG[:].rearrange("p k f -> p f k"),
                        mybir.AxisListType.X, ALU.add)
                return TT

            # ================= stage A: locally-connected layers =============
            XP = cpool.tile([28, 8, 32, 32], bf, tag="XP")
            nc.sync.dma_start(XP[:], xp_d[:])

            # patch buffers: partitions (kx*32+c) plus ones-row at 96
            P1 = apool.tile([97, 8, 32, 32], bf, tag="P1")   # LC1 out patches
            P2 = apool.tile([97, 6, 32, 30], bf, tag="P2")   # LC2 out patches
            nc.vector.memset(P2[0:32, :, :, 28:30], 0.0)     # x-pad cols only
            nc.vector.memset(P1[96:97, :, :, :], 1.0)
            nc.vector.memset(P2[96:97, :, :, :], 1.0)
            # LC3 output, laid out for the AllToAll: [o, j(dest core), b, y, x]
            ACT3 = apool.tile([32, 8, 4, 4, 28], bf, tag="ACT3")

            # ---- LC1: out rows 0..8 (local), 32 x-locs (30 true + 2 pad) ----
            for yb in range(4):
                W1t = wpool.tile([28, 2, 8, 128], bf, tag="wrow")
                nc.sync.dma_start(W1t[:], w1p_d[:, 2 * yb:2 * yb + 2])
                PS = pspool.tile([128, 2, 8, 32], f32, tag="ps")
                for gi in range(16):
                    y, g = 2 * yb + gi // 8, gi % 8
                    for li in range(4):
                        nc.tensor.matmul(
                            PS[32 * li:32 * li + 32, gi // 8, g, :],
                            W1t[:, gi // 8, g, 32 * li:32 * li + 32],
                            XP[:, y, :, 4 * g + li],
                            start=True, stop=True,
                            tile_position=(0, 32 * li),
                        )
                for g2 in range(4):
                    nc.scalar.activation(
                        P1[0:32, 2 * yb:2 * yb + 2, :, g2::4]
                          .rearrange("p y b x -> p y x b"),
                        PS[32 * g2:32 * g2 + 32, :, :, :],
                        AF.Prelu, alpha=ALPHA,
                    )
                # x-shifted replicas for partition blocks 1, 2
                nc.vector.tensor_copy(
                    P1[32:64, 2 * yb:2 * yb + 2, :, 0:31],
                    P1[0:32, 2 * yb:2 * yb + 2, :, 1:32])
                nc.vector.tensor_copy(
                    P1[64:96, 2 * yb:2 * yb + 2, :, 0:30],
                    P1[0:32, 2 * yb:2 * yb + 2, :, 2:32])

            # ---- LC2: 6 local rows, 28 x-locs (7 groups exactly) ----
            for y in range(6):
                W2t = wpool.tile([97, 7, 3, 128], bf, tag="wrow")
                nc.sync.dma_start(W2t[:], w2p_d[y])
                PS = pspool.tile([128, 7, 32], f32, tag="ps")
                for g in range(7):
                    for ky in range(3):
                        for li in range(4):
                            nc.tensor.matmul(
                                PS[32 * li:32 * li + 32, g, :],
                                W2t[:, g, ky, 32 * li:32 * li + 32],
                                P1[:, y + ky, :, 4 * g + li],
                                start=(ky == 0), stop=(ky == 2),
                                tile_position=(0, 32 * li),
                            )
                for g2 in range(4):
                    nc.scalar.activation(
                        P2[0:32, y, :, g2:g2 + 25:4].rearrange("p b x -> p x b"),
                        PS[32 * g2:32 * g2 + 32, :, :],
                        AF.Prelu, alpha=ALPHA,
                    )
                nc.vector.tensor_copy(P2[32:64, y, :, 0:28], P2[0:32, y, :, 1:29])
                nc.vector.tensor_copy(P2[64:96, y, :, 0:28], P2[0:32, y, :, 2:30])

            # ---- LC3: 4 local rows, 28 x-locs (26 true + 2 zero-padded) ----
            for y in range(4):
                W3t = wpool.tile([97, 7, 3, 128], bf, tag="wrow")
                nc.sync.dma_start(W3t[:], w3p_d[y])
                PS = pspool.tile([128, 7, 32], f32, tag="ps")
                for g in range(7):
                    for ky in range(3):
                        for li in range(4):
                            nc.tensor.matmul(
                                PS[32 * li:32 * li + 32, g, :],
                                W3t[:, g, ky, 32 * li:32 * li + 32],
                                P2[:, y + ky, :, 4 * g + li],
                                start=(ky == 0), stop=(ky == 2),
                                tile_position=(0, 32 * li),
                            )
                for g2 in range(4):
                    nc.scalar.activation(
                        ACT3[0:32, :, :, y, g2::4].rearrange("p j b x -> p x j b"),
                        PS[32 * g2:32 * g2 + 32, :, :],
                        AF.Prelu, alpha=ALPHA,
                    )

            # ============== transition: AllToAll to batch sharding ===========
            # conv1 input patches: [kx*32+c, b, ypad28, xpad28]; zero only the
            # 1px border of block 0, interior is overwritten
            PB1 = apool.tile([96, 4, 28, 28], bf, tag="P1")
            nc.vector.memset(PB1[0:32, :, 0:1, :], 0.0)
            nc.vector.memset(PB1[0:32, :, 27:28, :], 0.0)
            nc.vector.memset(PB1[0:32, :, 1:27, 0:1], 0.0)
            nc.vector.memset(PB1[0:32, :, 1:27, 27:28], 0.0)
            if USE_RDMA_A2A:
                # repack to 128 partitions (32*b + och); descriptors were
                # prepped above, data is read when the trigger fires
                for b in range(4):
                    nc.sync.dma_start(A2S[32 * b:32 * b + 32, :, :, :],
                                      ACT3[:, :, b, :, :])
                nc.gpsimd.trigger_dma(count=None, queue_num=0,
                                      signals_writable=[A2S[:], PBQ2[:]])
                for b in range(4):
                    cp = nc.vector.tensor_copy(
                        PB1[0:32, b, 1:27, 1:27],
                        PBQ2[32 * b:32 * b + 32, :, 0:26])
                    cp.wait_op(a2a_rsem, wreg, "sem-ge")
                rel = nc.vector.sem_inc(q0free, 1)
                rel.wait_op(a2a_rsem, wreg, "sem-ge")
            else:
                a2a_in = dpool.tile([8, 32, 4, 4, 28], bf, tag="a2a_in")
                a2a_out = dpool.tile([8, 32, 4, 4, 28], bf, tag="a2a_out")
                nc.gpsimd.dma_start(
                    a2a_in[:].rearrange("j o b y x -> o j (b y x)"),
                    ACT3[:].rearrange("p j b y x -> p j (b y x)"))
                nc.gpsimd.collective_compute(
                    "AllToAll", ALU.bypass, replica_groups=[CORES],
                    ins=[a2a_in.opt()], outs=[a2a_out.opt()])
                # one bulk DMA for the whole A2A result, then cheap DVE
                # row-selection copies (8 small DMAs each pay ~1.5us of DGE
                # + semaphore latency, serializing ~15us after the A2A)
                PBA = apool.tile([32, 8, 4, 4, 28], bf, tag="ACT3")
                nc.gpsimd.dma_start(
                    PBA[:], a2a_out[:].rearrange("i o b y x -> o i b y x"))
                for i in range(NCORES):
                    lo, hi = TAKE[i]
                    nc.vector.tensor_copy(
                        PB1[0:32, :, 1 + lo:1 + hi, 1:27],
                        PBA[:, i, :, lo - ST[i]:hi - ST[i], 0:26])
            nc.vector.tensor_copy(PB1[32:64, :, :, 0:27], PB1[0:32, :, :, 1:28])
            nc.vector.tensor_copy(PB1[64:96, :, :, 0:26], PB1[0:32, :, :, 2:28])
            if USE_RDMA_A2A:
                emit_stat_preps(4)

            # eps tile for sqrt(var + eps)
            EPST = spool.tile([128, 1], f32, tag="EPST")
            nc.vector.memset(EPST[:], EPS)

            def bn_scale_shift(tag, C, TTs, TTq, bn_g, bn_b, n_elems):
                """scale/shift from total sum TTs / sumsq TTq ([C,1] views)."""
                MEAN = spool.tile([C, 1], f32, tag=f"MEAN{tag}")
                MSQ = spool.tile([C, 1], f32, tag=f"MSQ{tag}")
                VAR = spool.tile([C, 1], f32, tag=f"VAR{tag}")
                SD = spool.tile([C, 1], f32, tag=f"SD{tag}")
                SC = spool.tile([C, 1], f32, tag=f"SC{tag}")
                TB = spool.tile([C, 1], f32, tag=f"TB{tag}")
                nc.scalar.mul(MEAN[:], TTs, 1.0 / n_elems)
                nc.scalar.mul(MSQ[:], TTq, 1.0 / n_elems)
                nc.vector.tensor_mul(VAR[:], MEAN[:], MEAN[:])
                nc.vector.tensor_sub(VAR[:], MSQ[:], VAR[:])
                nc.scalar.activation(SD[:], VAR[:], AF.Sqrt, bias=EPST[0:C, :])
                nc.vector.reciprocal(SD[:], SD[:])
                nc.vector.tensor_mul(SC[:], bn_g, SD[:])
                nc.vector.tensor_mul(TB[:], MEAN[:], SC[:])
                nc.vector.tensor_sub(TB[:], bn_b, TB[:])
                return SC, TB

            # ======================= conv1 + BN + pool =======================
            W1B = cpool.tile([96, 3, 64], bf, tag="W1B")
            nc.sync.dma_start(W1B[:], w1b_d[:])
            BN1 = cpool.tile([64, 2], f32, tag="BN1")
            nc.sync.dma_start(BN1[:], bn1_d[:])
            O1 = apool.tile([64, 4, 2, 13, 26], f32, tag="P2")  # (b, yh, y13, x26)
            SA1 = spool.tile([64, 8], f32, tag="SA1")
            QA1 = spool.tile([64, 8], f32, tag="QA1")
            for nb in range(8):
                b, yh = nb // 2, nb % 2
                PS = pspool.tile([64, 13, 26], f32, tag="ps")
                for ky in range(3):
                    nc.tensor.matmul(
                        PS[:], W1B[:, ky, :],
                        PB1[0:96, b, 13 * yh + ky:13 * yh + ky + 13, 0:26],
                        start=(ky == 0), stop=(ky == 2))
                nc.scalar.activation(O1[:, b, yh, :, :], PS[:], AF.Copy,
                                     accum_out=SA1[:, nb:nb + 1])
                SCR = scrpool.tile([64, 13, 26], f32, tag="scr")
                nc.scalar.activation(SCR[:], O1[:, b, yh, :, :], AF.Square,
                                     accum_out=QA1[:, nb:nb + 1])
            nc.vector.memset(SND[1][64:128, :], 0.0)
            nc.vector.tensor_reduce(SND[1][0:64, 0:1], SA1[:],
                                    mybir.AxisListType.X, ALU.add)
            nc.vector.tensor_reduce(SND[1][0:64, 1:2], QA1[:],
                                    mybir.AxisListType.X, ALU.add)
            TT1 = allreduce_stats(1)
            SC1, TB1 = bn_scale_shift("1", 64, TT1[0:64, 0:1], TT1[0:64, 1:2],
                                      BN1[:, 0:1], BN1[:, 1:2], N1)
            PA = apool.tile([64, 4, 26, 13], f32, tag="PA")
            T1 = O1[:].rearrange("p b h y x -> p b (h y) x")
            for bh in range(2):
                nc.scalar.activation(O1[:, 2 * bh:2 * bh + 2], 
                                     O1[:, 2 * bh:2 * bh + 2], AF.Prelu,
                                     bias=TB1[:], scale=SC1[:], alpha=ALPHA)
                nc.vector.tensor_max(PA[:, 2 * bh:2 * bh + 2],
                                     T1[:, 2 * bh:2 * bh + 2, :, 0::2],
                                     T1[:, 2 * bh:2 * bh + 2, :, 1::2])
            PB2a = apool.tile([128, 4, 15, 15], bf, tag="ACT3")
            PB2b = apool.tile([64, 4, 15, 15], bf, tag="PB2b")
            nc.vector.memset(PB2a[0:64, :, 0:1, :], 0.0)
            nc.vector.memset(PB2a[0:64, :, 14:15, :], 0.0)
            nc.vector.memset(PB2a[0:64, :, 1:14, 0:1], 0.0)
            nc.vector.memset(PB2a[0:64, :, 1:14, 14:15], 0.0)
            nc.vector.tensor_max(PB2a[0:64, :, 1:14, 1:14],
                                 PA[:, :, 0:26:2, :], PA[:, :, 1:26:2, :])
            nc.vector.tensor_copy(PB2a[64:128, :, :, 0:14], PB2a[0:64, :, :, 1:15])
            nc.vector.tensor_copy(PB2b[0:64, :, :, 0:13], PB2a[0:64, :, :, 2:15])

            # ======================= conv2 + BN + pool =======================
            W2BA = cpool.tile([128, 3, 128], bf, tag="W2BA")
            nc.sync.dma_start(W2BA[:], w2ba_d[:])
            W2BB = cpool.tile([64, 3, 128], bf, tag="W2BB")
            nc.sync.dma_start(W2BB[:], w2bb_d[:])
            BN2 = cpool.tile([128, 2], f32, tag="BN2")
            nc.sync.dma_start(BN2[:], bn2_d[:])
            O2 = apool.tile([128, 4, 13, 13], f32, tag="O2")
            SA2 = spool.tile([128, 4], f32, tag="SA2")
            QA2 = spool.tile([128, 4], f32, tag="QA2")
            for b in range(4):
                PS = pspool.tile([128, 13, 13], f32, tag="ps")
                for ky in range(3):
                    nc.tensor.matmul(PS[:], W2BA[:, ky, :],
                                     PB2a[:, b, ky:ky + 13, 0:13],
                                     start=(ky == 0), stop=False)
                for ky in range(3):
                    nc.tensor.matmul(PS[:], W2BB[:, ky, :],
                                     PB2b[:, b, ky:ky + 13, 0:13],
                                     start=False, stop=(ky == 2))
                nc.scalar.activation(O2[:, b, :, :], PS[:], AF.Copy,
                                     accum_out=SA2[:, b:b + 1])
                SCR = scrpool.tile([128, 13, 13], f32, tag="scr")
                nc.scalar.activation(SCR[:], O2[:, b, :, :], AF.Square,
                                     accum_out=QA2[:, b:b + 1])
            nc.vector.tensor_reduce(SND[2][:, 0:1], SA2[:],
                                    mybir.AxisListType.X, ALU.add)
            nc.vector.tensor_reduce(SND[2][:, 1:2], QA2[:],
                                    mybir.AxisListType.X, ALU.add)
            TT2 = allreduce_stats(2)
            SC2, TB2 = bn_scale_shift("2", 128, TT2[:, 0:1], TT2[:, 1:2],
                                      BN2[:, 0:1], BN2[:, 1:2], N2)
            PA2 = apool.tile([128, 4, 12, 6], f32, tag="PA2")
            for bh in range(2):
                nc.scalar.activation(O2[:, 2 * bh:2 * bh + 2],
                                     O2[:, 2 * bh:2 * bh + 2], AF.Prelu,
                                     bias=TB2[:], scale=SC2[:], alpha=ALPHA)
                nc.vector.tensor_max(PA2[:, 2 * bh:2 * bh + 2],
                                     O2[:, 2 * bh:2 * bh + 2, 0:12, 0:12:2],
                                     O2[:, 2 * bh:2 * bh + 2, 0:12, 1:13:2])
            PB3a = apool.tile([128, 4, 8, 8], bf, tag="P1")
            PB3b = apool.tile([128, 4, 8, 8], bf, tag="PB3b")
            PB3c = apool.tile([128, 4, 8, 8], bf, tag="PB3c")
            nc.vector.memset(PB3a[:, :, 0:1, :], 0.0)
            nc.vector.memset(PB3a[:, :, 7:8, :], 0.0)
            nc.vector.memset(PB3a[:, :, 1:7, 0:1], 0.0)
            nc.vector.memset(PB3a[:, :, 1:7, 7:8], 0.0)
            nc.vector.tensor_max(PB3a[:, :, 1:7, 1:7],
                                 PA2[:, :, 0:12:2, :], PA2[:, :, 1:12:2, :])
            nc.vector.tensor_copy(PB3b[:, :, :, 0:7], PB3a[:, :, :, 1:8])
            nc.vector.tensor_copy(PB3c[:, :, :, 0:6], PB3a[:, :, :, 2:8])

            # ======================= conv3 + BN + pool =======================
            W3B = cpool.tile([128, 3, 3, 256], bf, tag="W3B")
            nc.sync.dma_start(W3B[:], w3b_d[:])
            BN3 = cpool.tile([128, 2, 2], f32, tag="BN3")
            nc.sync.dma_start(BN3[:], bn3_d[:])
            O3 = apool.tile([128, 2, 4, 6, 6], f32, tag="O3")  # (mh, b, y, x)
            SA3 = spool.tile([128, 2], f32, tag="SA3")
            QA3 = spool.tile([128, 2], f32, tag="QA3")
            PBs = [PB3a, PB3b, PB3c]
            for mh in range(2):
                PS = pspool.tile([128, 4, 6, 6], f32, tag="ps")
                for ky in range(3):
                    for kx in range(3):
                        nc.tensor.matmul(
                            PS[:], W3B[:, ky, kx, 128 * mh:128 * mh + 128],
                            PBs[kx][:, :, ky:ky + 6, 0:6],
                            start=(ky == 0 and kx == 0), stop=(ky == 2 and kx == 2))
                nc.scalar.activation(O3[:, mh, :, :, :], PS[:], AF.Copy,
                                     accum_out=SA3[:, mh:mh + 1])
                SCR = scrpool.tile([128, 4, 6, 6], f32, tag="scr")
                nc.scalar.activation(SCR[:], O3[:, mh, :, :, :], AF.Square,
                                     accum_out=QA3[:, mh:mh + 1])
            nc.vector.tensor_copy(SND[3][:, 0:4:2], SA3[:])
            nc.vector.tensor_copy(SND[3][:, 1:4:2], QA3[:])
            TT3 = allreduce_stats(3)
            SC3, TB3 = {}, {}
            for mh in range(2):
                SC3[mh], TB3[mh] = bn_scale_shift(
                    f"3{mh}", 128, TT3[:, 2 * mh:2 * mh + 1],
                    TT3[:, 2 * mh + 1:2 * mh + 2],
                    BN3[:, mh, 0:1], BN3[:, mh, 1:2], N3)
            PB4 = []
            for kx in range(3):
                row = []
                for mh in range(2):
                    pb4t = apool.tile([128, 4, 5, 5], bf, tag=f"PB4{kx}{mh}")
                    row.append(pb4t)
                PB4.append(row)
            for mh in range(2):
                nc.scalar.activation(O3[:, mh, :, :, :], O3[:, mh, :, :, :],
                                     AF.Prelu, bias=TB3[mh][:], scale=SC3[mh][:],
                                     alpha=ALPHA)
                T3 = O3[:, mh, :, :, :]
                PA3 = apool.tile([128, 4, 6, 3], f32, tag=f"PA3{mh}")
                nc.vector.tensor_max(PA3[:], T3[:, :, :, 0::2], T3[:, :, :, 1::2])
                nc.vector.memset(PB4[0][mh][:, :, 0:1, :], 0.0)
                nc.vector.memset(PB4[0][mh][:, :, 4:5, :], 0.0)
                nc.vector.memset(PB4[0][mh][:, :, 1:4, 0:1], 0.0)
                nc.vector.memset(PB4[0][mh][:, :, 1:4, 4:5], 0.0)
                nc.vector.tensor_max(PB4[0][mh][:, :, 1:4, 1:4],
                                     PA3[:, :, 0:6:2, :], PA3[:, :, 1:6:2, :])
                nc.vector.tensor_copy(PB4[1][mh][:, :, :, 0:4],
                                      PB4[0][mh][:, :, :, 1:5])
                nc.vector.tensor_copy(PB4[2][mh][:, :, :, 0:3],
                                      PB4[0][mh][:, :, :, 2:5])

            # ======================= convf + BN + tanh =======================
            WFB = cpool.tile([128, 2, 3, 3, 256], bf, tag="WFB")
            nc.sync.dma_start(WFB[:], wfb_d[:])
            BNF = cpool.tile([128, 2, 2], f32, tag="BNF")
            nc.sync.dma_start(BNF[:], bnf_d[:])
            OF = apool.tile([128, 2, 4, 3, 3], f32, tag="OF")
            SAF = spool.tile([128, 2], f32, tag="SAF")
            QAF = spool.tile([128, 2], f32, tag="QAF")
            for mh in range(2):
                PS = pspool.tile([128, 4, 3, 3], f32, tag="ps")
                first = True
                for cb in range(2):
                    for ky in range(3):
                        for kx in range(3):
                            nc.tensor.matmul(
                                PS[:], WFB[:, cb, ky, kx, 128 * mh:128 * mh + 128],
                                PB4[kx][cb][:, :, ky:ky + 3, 0:3],
                                start=first, stop=(cb == 1 and ky == 2 and kx == 2))
                            first = False
                nc.scalar.activation(OF[:, mh, :, :, :], PS[:], AF.Copy,
                                     accum_out=SAF[:, mh:mh + 1])
                SCR = scrpool.tile([128, 4, 3, 3], f32, tag="scr")
                nc.scalar.activation(SCR[:], OF[:, mh, :, :, :], AF.Square,
                                     accum_out=QAF[:, mh:mh + 1])
            nc.vector.tensor_copy(SND[4][:, 0:4:2], SAF[:])
            nc.vector.tensor_copy(SND[4][:, 1:4:2], QAF[:])
            TTF = allreduce_stats(4)
            for mh in range(2):
                SCt, TBt = bn_scale_shift(
                    f"f{mh}", 128, TTF[:, 2 * mh:2 * mh + 1],
                    TTF[:, 2 * mh + 1:2 * mh + 2],
                    BNF[:, mh, 0:1], BNF[:, mh, 1:2], NF)
                OUTT = apool.tile([128, 4, 3, 3], bf, tag=f"OUTT{mh}")
                nc.scalar.activation(OUTT[:], OF[:, mh, :, :, :], AF.Tanh,
                                     bias=TBt[:], scale=SCt[:])
                nc.sync.dma_start(
                    out_d[:, 128 * mh:128 * mh + 128, :, :]
                        .rearrange("b c y x -> c b y x"),
                    OUTT[:])
    return nc


def _prep(inputs):
    """Host-side shard + layout prep. Pure data movement (plus dtype cast)."""
    f32 = np.float32
    x = np.asarray(inputs["x"], f32)
    lc1_w = np.asarray(inputs["lc1_w"], f32)[0]  # (32,3,30,30,9)
    lc1_b = np.asarray(inputs["lc1_b"], f32)[0]  # (32,30,30)
    lc2_w = np.asarray(inputs["lc2_w"], f32)[0]  # (32,32,28,28,9)
    lc2_b = np.asarray(inputs["lc2_b"], f32)[0]
    lc3_w = np.asarray(inputs["lc3_w"], f32)[0]  # (32,32,26,26,9)
    lc3_b = np.asarray(inputs["lc3_b"], f32)[0]

    # replicated stage-B weights
    c1w = np.asarray(inputs["c1_w"], f32)
    c2w = np.asarray(inputs["c2_w"], f32)
    c3w = np.asarray(inputs["c3_w"], f32)
    cfw = np.asarray(inputs["cf_w"], f32)
    w1b = np.ascontiguousarray(c1w.transpose(3, 1, 2, 0).reshape(96, 3, 64)).astype(CAST)
    w2ba = np.ascontiguousarray(
        c2w[:, :, :, 0:2].transpose(3, 1, 2, 0).reshape(128, 3, 128)).astype(CAST)
    w2bb = np.ascontiguousarray(c2w[:, :, :, 2].transpose(1, 2, 0)).astype(CAST)
    w3b = np.ascontiguousarray(c3w.transpose(1, 2, 3, 0)).astype(CAST)
    wfb = np.ascontiguousarray(
        cfw.reshape(256, 2, 128, 3, 3).transpose(2, 1, 3, 4, 0)).astype(CAST)
    bn1 = np.stack([np.asarray(inputs["c1_g"], f32),
                    np.asarray(inputs["c1_beta"], f32)], axis=1)
    bn2 = np.stack([np.asarray(inputs["c2_g"], f32),
                    np.asarray(inputs["c2_beta"], f32)], axis=1)
    bn3 = np.stack([np.asarray(inputs["c3_g"], f32).reshape(2, 128).T,
                    np.asarray(inputs["c3_beta"], f32).reshape(2, 128).T], axis=2)
    bnf = np.stack([np.asarray(inputs["cf_g"], f32).reshape(2, 128).T,
                    np.asarray(inputs["cf_beta"], f32).reshape(2, 128).T], axis=2)

    def lc_pack(wsl, bsl, nrow, width):
        """wsl: (32o,32c,nrow,width,9) -> (nrow, 97, G, 3, 128); bsl: (32o,nrow,width)"""
        G = 7
        wp = np.zeros((32, 32, nrow, 4 * G, 9), f32)
        wp[:, :, :, :width] = wsl
        bp = np.zeros((32, nrow, 4 * G), f32)
        bp[:, :, :width] = bsl
        arr = wp.reshape(32, 32, nrow, G, 4, 3, 3)  # o c y g li ky kx
        arr = arr.transpose(2, 6, 1, 3, 5, 4, 0).reshape(nrow, 96, G, 3, 128)
        outw = np.zeros((nrow, 97, G, 3, 128), f32)
        outw[:, :96] = arr
        outw[:, 96, :, 0, :] = bp.transpose(1, 2, 0).reshape(nrow, G, 4, 32)\
                                 .reshape(nrow, G, 128)
        return outw.astype(CAST)

    # core c computes LC3 rows ST[lab[c]] and stage-B images [4*lab[c]..);
    # with the rdma A2A, lab is the XOR label from discovery, else identity
    lab = _discover_dmap() if USE_RDMA_A2A else list(range(NCORES))

    in_maps = []
    xpad = np.zeros((32, 3, 32, 34), f32)
    xpad[:, :, :, :32] = x
    for c in range(NCORES):
        s = ST[lab[c]]
        if USE_RDMA_A2A:
            # batch-group permutation: XP group m holds image group lab^m,
            # so A2A slot m always carries the receiver's own images
            border = np.concatenate(
                [np.arange(4 * (lab[c] ^ m), 4 * (lab[c] ^ m) + 4)
                 for m in range(8)])
        else:
            border = np.arange(32)
        xb = xpad[border]
        xp = np.zeros((28, 8, 32, 32), f32)
        for ky in range(3):
            for kx in range(3):
                k = ky * 3 + kx
                blk = xb[:, :, s + ky:s + ky + 8, kx:kx + 32]  # (b,c,y,x)
                xp[3 * k:3 * k + 3] = blk.transpose(1, 2, 0, 3)
        xp[27] = 1.0

        w1sl = np.zeros((32, 3, 8, 32, 9), f32)
        w1sl[:, :, :, :30] = lc1_w[:, :, s:s + 8]
        b1sl = np.zeros((32, 8, 32), f32)
        b1sl[:, :, :30] = lc1_b[:, s:s + 8]
        arr = w1sl.reshape(32, 3, 8, 8, 4, 9)  # o c y g li k
        arr = arr.transpose(5, 1, 2, 3, 4, 0).reshape(27, 8, 8, 128)
        w1p = np.zeros((28, 8, 8, 128), f32)
        w1p[:27] = arr
        w1p[27] = b1sl.transpose(1, 2, 0).reshape(8, 8, 4, 32).reshape(8, 8, 128)

        w2p = lc_pack(lc2_w[:, :, s:s + 6], lc2_b[:, s:s + 6], 6, 28)
        w3p = lc_pack(lc3_w[:, :, s:s + 4], lc3_b[:, s:s + 4], 4, 26)

        in_maps.append({
            "xp": xp.astype(CAST), "w1p": w1p.astype(CAST),
            "w2p": w2p, "w3p": w3p,
            "w1b": w1b, "w2ba": w2ba, "w2bb": w2bb, "w3b": w3b, "wfb": wfb,
            "bn1": bn1, "bn2": bn2, "bn3": bn3, "bnf": bnf,
            "myrow": np.array([[ST[lab[c]] * 28]], np.uint32),
        })
    return in_maps


def get_nc():
    if "nc" not in _cache:
        nc = _build()
        nc.compile()
        _cache["nc"] = nc
    return _cache["nc"]


class Runner:
    """Persistent SPMD executor for the compiled NEFF.

    run_bass_kernel_spmd rebuilds its jit closure (retrace + relower,
    ~0.6s) and re-transfers every input (~60MB over the axon tunnel,
    ~1.1s) on EVERY call. This runner builds the sharded jit ONCE,
    uploads inputs ONCE (device-resident; one batched device_put), and
    then each execution is just dispatch + output fetch. The NEFF's
    donated-output workaround is replaced by persistent dummy output
    buffers: the kernel writes every element of `out`, so the zero-init
    (only needed for unwritten bytes) is unnecessary, and without jit
    donation the buffers survive across calls (verified bit-identical
    against the run_bass_kernel_spmd path).
    """

    def __init__(self, nc):
        import jax
        from jax.sharding import Mesh, PartitionSpec, NamedSharding
        from jax.experimental.shard_map import shard_map

        self.jax = jax
        bass2jax.install_neuronx_cc_hook()
        assert nc.dbg_addr is None or not nc.dbg_callbacks
        pname = nc.partition_id_tensor.name if nc.partition_id_tensor else None
        in_names, out_names, out_avals = [], [], []
        for alloc in nc.m.functions[0].allocations:
            if not isinstance(alloc, mybir.MemoryLocationSet):
                continue
            name = alloc.memorylocations[0].name
            if alloc.kind == "ExternalInput":
                if name != pname:
                    in_names.append(name)
            elif alloc.kind == "ExternalOutput":
                shape = tuple(alloc.tensor_shape)
                dtype = mybir.dt.np(alloc.dtype)
                out_names.append(name)
                out_avals.append(jax.core.ShapedArray(shape, dtype))
        self.in_names, self.out_names, self.out_avals = in_names, out_names, out_avals
        in_names_all = in_names + out_names + ([pname] if pname else [])

        def _body(*args):
            operands = list(args)
            if pname is not None:
                operands.append(bass2jax.partition_id_tensor())
            return tuple(bass2jax._bass_exec_p.bind(
                *operands, out_avals=tuple(out_avals),
                in_names=tuple(in_names_all), out_names=tuple(out_names),
                lowering_input_output_aliases=(), sim_require_finite=True,
                sim_require_nnan=True, nc=nc))

        devices = jax.devices()[:NCORES]
        assert len(devices) == NCORES
        mesh = Mesh(np.asarray(devices), ("core",))
        self.sharding = NamedSharding(mesh, PartitionSpec("core"))
        n_all = len(in_names) + len(out_names)
        self.fn = jax.jit(
            shard_map(_body, mesh=mesh,
                      in_specs=(PartitionSpec("core"),) * n_all,
                      out_specs=(PartitionSpec("core"),) * len(out_names),
                      check_rep=False),
            keep_unused=True)
        # persistent (non-donated) buffers for the NEFF output slots
        self.out_bufs = jax.device_put(
            [np.zeros((NCORES * av.shape[0], *av.shape[1:]), av.dtype)
             for av in out_avals], self.sharding)

    def upload(self, in_maps):
        """Concat per-core inputs and transfer once; returns device args."""
        concat = [np.concatenate([np.asarray(in_maps[c][nm])
                                  for c in range(NCORES)], axis=0)
                  for nm in self.in_names]
        dev_in = self.jax.device_put(concat, self.sharding)
        self.jax.block_until_ready(dev_in)
        return dev_in

    def submit(self, dev_in):
        """Dispatch one SPMD execution (async); returns output handles."""
        return self.fn(*dev_in, *self.out_bufs)

    def fetch(self, handles):
        """Materialize one execution's outputs to numpy (full shape)."""
        for h in handles:
            h.copy_to_host_async()
        return [np.asarray(h) for h in handles]


def get_runner():
    if "runner" not in _cache:
        _cache["runner"] = Runner(get_nc())
    return _cache["runner"]


def kernel(**inputs) -> np.ndarray:
    r = get_runner()
    in_maps = _prep(inputs)
    lab = _discover_dmap() if USE_RDMA_A2A else list(range(NCORES))
    dev_in = r.upload(in_maps)
    res = r.fetch(r.submit(dev_in))[0]  # (NCORES*4, 256, 3, 3)
    out = np.empty((32, 256, 3, 3), np.float32)
    for c in range(NCORES):
        out[4 * lab[c]:4 * lab[c] + 4] = res[4 * c:4 * c + 4].astype(np.float32)
    return out



# revision 6
# speedup vs baseline: 392.0367x; 18.2706x over previous
"""Trainium2 Bass kernel for nn_Locally_Connected_Module.

Network: 3 locally-connected 3x3 layers (per-location weights, ~57MB total),
then 4 conv3x3+BN(+PReLU/tanh) blocks with 3 maxpools.
  x (32,3,32,32) -> LC1 -> (32,32,30,30) -> LC2 -> (32,32,28,28) -> LC3 ->
  (32,32,26,26) -> conv1+bn+prelu+pool -> (32,64,13,13) -> conv2.. ->
  (32,128,6,6) -> conv3.. -> (32,256,3,3) -> convf+bn+tanh -> (32,256,3,3)

Sharding:
  Stage A (LC layers): SPATIAL row-sharding over the 8 cores; each core
  computes a 4-row slice of LC3 output (with halo back through LC2/LC1) for
  the FULL batch, so each core only reads ~1/8 of the per-location LC
  weights. LC bias is folded in as a K=97th "ones" row.
  Transition: AllToAll converts (all batch, row slice) -> (4 images,
  all rows) per core.
  Stage B (convs): batch-parallel, 4 images/core. Train-mode BN batch stats
  are reduced cross-core with small AllGathers (partial sum/sumsq per core,
  summed locally after the gather). Conv biases are skipped: train-mode BN
  makes them no-ops. Final output is batch-sharded; host concatenates.

Compute dtype: fp16 operands (PE matmul is 4x faster than fp32 and fp16
keeps ~11 mantissa bits vs bf16's 8, landing at ~7e-3 max rel err) with
fp32 PSUM accumulation and fp32 BN statistics math. Memsets only touch pad
borders; patch-shift replicas are width-trimmed to the consumed columns.

Experimental (env-gated OFF by default): KERNEL_RDMA/KERNEL_A2A switch the
BN-stat reductions and the AllToAll to XOR-slot remote_dma_broadcast
exchanges (~2us vs the ~15us fixed cost of each collective). The stats
exchange and the A2A each verify standalone (see probe_rdma*.py: XOR-slot
routing, symbolic row-offset out_ap, register-valued remote-sem waits),
but the combined full kernel still hits an opaque device fault (ruled
out: tile-pool reuse racing remote writes -- receive buffers are raw SBUF
tensors now; SWDGE ring overflow -- each broadcast prep is 66 descriptors
vs a 1024/queue ring, and raising dynamic_dma_scratch_size did not help).
collective_compute + remote_dma also cannot coexist in one NEFF (hangs),
so the verified collective path ships.
"""
import numpy as np

import concourse.bass as bass
import concourse.bacc as bacc
import concourse.mybir as mybir
import concourse.tile as tile
from concourse.bass_utils import run_bass_kernel_spmd
from concourse import bass2jax

dt = mybir.dt
AF = mybir.ActivationFunctionType
ALU = mybir.AluOpType

import os
USE_FP16 = os.environ.get("KERNEL_FP16", "1") == "1"
# bitmask: bit (li-1) set -> BN layer li uses the remote_dma stat exchange;
# clear -> collective AllGather fallback
RDMA_MASK = int(os.environ.get("KERNEL_RDMA", "0"))
# AllToAll via remote_dma (Switch on the core's XOR label) vs collective.
# NOTE: collective_compute and remote_dma hang when mixed in one NEFF, so
# A2A=1 requires RDMA_MASK=15 and A2A=0 requires RDMA_MASK=0.
USE_RDMA_A2A = os.environ.get("KERNEL_A2A", "0") == "1"
# debug bitmask: bit (li-1) -> use local-only stats for BN layer li
LOCAL_STATS = int(os.environ.get("KERNEL_LOCALSTATS", "0"))
CAST = np.float16 if USE_FP16 else np.float32

NCORES = 8
CORES = list(range(NCORES))
EPS = 1e-5
ALPHA = 0.25

# LC3 output row starts per core (each computes rows [s, s+4) of 26)
ST = [0, 4, 8, 11, 14, 17, 20, 22]
# which global rows to take from each core's chunk when reassembling
TAKE = [(0, 4), (4, 8), (8, 12), (12, 15), (15, 18), (18, 21), (21, 24), (24, 26)]

N1, N2, N3, NF = 32 * 26 * 26, 32 * 13 * 13, 32 * 6 * 6, 32 * 3 * 3

_cache = {}


def _build_discovery():
    """Tiny NEFF: each core XOR-slot-broadcasts its logical id; receivers'
    slot k then holds the logical id of the core whose physical NC id is
    (own_pid ^ k). Core 0's slots give each logical core's XOR label d:
    d[slots0[k]] = k. Needed because the driver's logical->physical NC map
    is not readable from the client."""
    nc = bacc.Bacc("TRN2", target_bir_lowering=False)
    f32 = dt.float32
    val_d = nc.dram_tensor("val", [128, 1], f32, kind="ExternalInput")
    slots_d = nc.dram_tensor("slots", [128, 8], f32, kind="ExternalOutput")
    wreg = nc.vector.alloc_register("rwait")
    nc.vector.reg_mov(wreg, 16)
    with tile.TileContext(nc) as tc:
        with tc.tile_pool(name="p", bufs=1) as pool:
            VAL = pool.tile([128, 1], f32, tag="VAL")
            nc.sync.dma_start(VAL[:], val_d[:])
            RECV = pool.tile([128, 8], f32, tag="RECV")
            rsem = nc.alloc_semaphore("rsem")
            lsem = nc.alloc_semaphore("lsem")
            for k in range(8):
                rd = [None] * 8
                rd[k] = (0, k)
                nc.gpsimd.remote_dma_broadcast(
                    RECV[:, k:k + 1], VAL[:],
                    remote_sem=rsem, local_sem=lsem, rdests=rd)
            nc.gpsimd.trigger_dma(count=None)
            SL = pool.tile([128, 8], f32, tag="SL")
            cp = nc.vector.tensor_copy(SL[:], RECV[:])
            cp.wait_op(rsem, wreg, "sem-ge")
            nc.sync.dma_start(slots_d[:], SL[:])
    return nc


def _discover_dmap():
    """Run the discovery NEFF once; return d[logical] = XOR label."""
    if "dmap" not in _cache:
        nc = _build_discovery()
        nc.compile()
        in_maps = [{"val": np.full((128, 1), float(c), np.float32)}
                   for c in range(NCORES)]
        res = run_bass_kernel_spmd(nc, in_maps, CORES)
        slots0 = res.results[0]["slots"][0]  # [8] sender logical ids
        d = [-1] * NCORES
        for k in range(NCORES):
            d[int(round(float(slots0[k])))] = k
        assert sorted(d) == list(range(NCORES)), f"bad discovery: {slots0}"
        _cache["dmap"] = d
    return _cache["dmap"]


def _build():
    nc = bacc.Bacc("TRN2", target_bir_lowering=False, num_swdge_queues=4)
    f32 = dt.float32
    bf = dt.float16 if USE_FP16 else dt.float32

    # ---- external inputs (per-core data, same shapes on all cores) ----
    xp_d = nc.dram_tensor("xp", [28, 8, 32, 32], bf, kind="ExternalInput")
    w1p_d = nc.dram_tensor("w1p", [28, 8, 8, 128], bf, kind="ExternalInput")
    w2p_d = nc.dram_tensor("w2p", [6, 97, 7, 3, 128], bf, kind="ExternalInput")
    w3p_d = nc.dram_tensor("w3p", [4, 97, 7, 3, 128], bf, kind="ExternalInput")
    w1b_d = nc.dram_tensor("w1b", [96, 3, 64], bf, kind="ExternalInput")
    w2ba_d = nc.dram_tensor("w2ba", [128, 3, 128], bf, kind="ExternalInput")
    w2bb_d = nc.dram_tensor("w2bb", [64, 3, 128], bf, kind="ExternalInput")
    w3b_d = nc.dram_tensor("w3b", [128, 3, 3, 256], bf, kind="ExternalInput")
    wfb_d = nc.dram_tensor("wfb", [128, 2, 3, 3, 256], bf, kind="ExternalInput")
    bn1_d = nc.dram_tensor("bn1", [64, 2], f32, kind="ExternalInput")
    bn2_d = nc.dram_tensor("bn2", [128, 2], f32, kind="ExternalInput")
    bn3_d = nc.dram_tensor("bn3", [128, 2, 2], f32, kind="ExternalInput")
    bnf_d = nc.dram_tensor("bnf", [128, 2, 2], f32, kind="ExternalInput")

    out_d = nc.dram_tensor("out", [4, 256, 3, 3], bf, kind="ExternalOutput")
    # per-core row offset ST[label]*28 for the A2A receive window
    myrow_d = nc.dram_tensor("myrow", [1, 1], dt.uint32, kind="ExternalInput")

    # register-valued wait target for remote-sem gates (preamble block so it
    # is set before any tile-scheduled instruction decodes a wait against it)
    wreg = nc.vector.alloc_register("rdma_wait16")
    nc.vector.reg_mov(wreg, 16)
    rowreg = nc.gpsimd.alloc_register("myrow_reg")
    nc.gpsimd.reg_load(rowreg, myrow_d[0:1, 0:1])
    rowval = nc.gpsimd.snap(rowreg, donate=True, min_val=0, max_val=22 * 28)

    # remote-write targets live OUTSIDE the tile pools: peers' RDMA writes
    # land asynchronously and must never race tile-pool buffer reuse
    FDIM = {1: 2, 2: 2, 3: 4, 4: 4}  # free f32 elems per layer tag
    RCV = {li: nc.alloc_sbuf_tensor(f"RCV{li}", [128, 8, FDIM[li]], f32)
           for li in (1, 2, 3, 4)}
    PBQ2 = nc.alloc_sbuf_tensor("PBQ2", [128, 26, 28], bf)

    with tile.TileContext(nc) as tc:
        with (
            tc.tile_pool(name="const", bufs=1) as cpool,
            tc.tile_pool(name="wrow", bufs=3) as wpool,
            tc.tile_pool(name="act", bufs=1) as apool,
            tc.tile_pool(name="stat", bufs=1) as spool,
            tc.tile_pool(name="scr", bufs=2) as scrpool,
            tc.tile_pool(name="psum", bufs=4, space="PSUM") as pspool,
            tc.tile_pool(name="dram", bufs=1, space="DRAM") as dpool,
        ):
            # ---- BN-stat exchange buffers + preps (hoisted; data deps are
            # deferred to each queue's trigger) ----
            # queue map: A2A shares q0 with BNf (BNf preps are emitted after
            # the A2A trigger, so the q0 ring order stays A2A -> BNf)
            QMAP = {1: 1, 2: 2, 3: 3, 4: 0} if USE_RDMA_A2A \
                else {1: 0, 2: 1, 3: 2, 4: 3}
            SND, RSEM = {}, {}
            lsem = nc.alloc_semaphore("rdma_lsem")
            for li in (1, 2, 3, 4):
                SND[li] = spool.tile([128, FDIM[li]], f32, tag=f"SND{li}",
                                     name=f"SND{li}")
                RSEM[li] = nc.alloc_semaphore(f"rsem{li}")

            def emit_stat_preps(li):
                if (LOCAL_STATS >> (li - 1)) & 1 or not (RDMA_MASK >> (li - 1)) & 1:
                    return
                for k in range(8):
                    rd = [None] * 8
                    rd[k] = (0, k)
                    prep = nc.gpsimd.remote_dma_broadcast(
                        RCV[li][:, k, :], SND[li][:],
                        remote_sem=RSEM[li], local_sem=lsem,
                        rdests=rd, queue_num=QMAP[li])
                    if USE_RDMA_A2A and QMAP[li] == 0:
                        # q0 is shared with the A2A: descriptor generation
                        # must not enter the ring before the A2A trigger has
                        # fired its 8 entries (ring is FIFO per queue)
                        prep.wait_op(q0free, 1, "sem-ge")

            # ---- A2A exchange buffers + preps (uniform across cores: the
            # host pre-permutes each core's batch groups so slot k always
            # carries the receiver's own images, and every sender writes its
            # full 4-row window — overlapping rows carry identical values.
            # Only the receive-row offset differs per core: a register offset
            # in the out_ap (symbolic AP), loaded from the myrow input.) ----
            a2a_rsem = nc.alloc_semaphore("a2a_rsem")
            q0free = nc.alloc_semaphore("q0free")
            if USE_RDMA_A2A:
                A2S = cpool.tile([128, 8, 4, 28], bf, tag="A2S", name="A2S")
                base = PBQ2[:, 0:4, :]
                out_sym = (bass.AP(base.tensor, base.offset + rowval, base.ap)
                           if os.environ.get("KERNEL_SYMOFF", "1") == "1"
                           else base)
                for k in range(8):
                    rd = [None] * 8
                    rd[k] = (0, k)
                    nc.gpsimd.remote_dma_broadcast(
                        out_sym, A2S[:, k, :, :],
                        remote_sem=a2a_rsem, local_sem=lsem,
                        rdests=rd, queue_num=0)

            for li in (1, 2, 3):
                emit_stat_preps(li)
            if not USE_RDMA_A2A:
                emit_stat_preps(4)

            def allreduce_stats(li):
                q = QMAP[li]
                """Cross-core sum of SND[li] -> TT [128, F].

                rdma path: trigger queue q's 8 slot-broadcasts; the trigger
                declares SND/RCV as writable signals so tile orders it after
                every SND writer (WAW; remote_dma preps carry no tile-visible
                data deps) and before the RCV slot-sum (RAW). The slot-sum
                additionally gates on the remote sem (register-valued target).
                Fallback: collective AllGather via DRAM bounce buffers."""
                F = FDIM[li]
                TT = spool.tile([128, F], f32, tag=f"TT{li}", name=f"TT{li}")
                if (LOCAL_STATS >> (li - 1)) & 1:
                    nc.vector.tensor_copy(TT[:], SND[li][:])
                elif (RDMA_MASK >> (li - 1)) & 1:
                    nc.gpsimd.trigger_dma(count=None, queue_num=q,
                                          signals_writable=[SND[li][:],
                                                            RCV[li][:]])
                    red = nc.vector.tensor_reduce(
                        TT[:], RCV[li][:].rearrange("p k f -> p f k"),
                        mybir.AxisListType.X, ALU.add)
                    red.wait_op(RSEM[li], wreg, "sem-ge")
                else:
                    sti = dpool.tile([128, F], f32, tag=f"sti{li}",
                                     name=f"sti{li}")
                    sto = dpool.tile([8, 128, F], f32, tag=f"sto{li}",
                                     name=f"sto{li}", addr_space="Shared")
                    nc.gpsimd.dma_start(sti[:], SND[li][:])
                    nc.gpsimd.collective_compute(
                        "AllGather", ALU.bypass, replica_groups=[CORES],
                        ins=[sti.opt()], outs=[sto.opt()])
                    SG = spool.tile([128, 8, F], f32, tag=f"SG{li}",
                                    name=f"SG{li}")
                    nc.gpsimd.dma_start(SG[:], sto[:].rearrange("i c f -> c i f"))
                    nc.vector.tensor_reduce(
                        TT[:], SG[:].rearrange("p k f -> p f k"),
                        mybir.AxisListType.X, ALU.add)
                return TT

            # ================= stage A: locally-connected layers =============
            XP = cpool.tile([28, 8, 32, 32], bf, tag="XP")
            nc.sync.dma_start(XP[:], xp_d[:])

            # patch buffers: partitions (kx*32+c) plus ones-row at 96
            P1 = apool.tile([97, 8, 32, 32], bf, tag="P1")   # LC1 out patches
            P2 = apool.tile([97, 6, 32, 30], bf, tag="P2")   # LC2 out patches
            nc.vector.memset(P2[0:32, :, :, 28:30], 0.0)     # x-pad cols only
            nc.vector.memset(P1[96:97, :, :, :], 1.0)
            nc.vector.memset(P2[96:97, :, :, :], 1.0)
            # LC3 output, laid out for the AllToAll: [o, j(dest core), b, y, x]
            ACT3 = apool.tile([32, 8, 4, 4, 28], bf, tag="ACT3")

            # ---- LC1: out rows 0..8 (local), 32 x-locs (30 true + 2 pad) ----
            for yb in range(4):
                W1t = wpool.tile([28, 2, 8, 128], bf, tag="wrow")
                nc.sync.dma_start(W1t[:], w1p_d[:, 2 * yb:2 * yb + 2])
                PS = pspool.tile([128, 2, 8, 32], f32, tag="ps")
                for gi in range(16):
                    y, g = 2 * yb + gi // 8, gi % 8
                    for li in range(4):
                        nc.tensor.matmul(
                            PS[32 * li:32 * li + 32, gi // 8, g, :],
                            W1t[:, gi // 8, g, 32 * li:32 * li + 32],
                            XP[:, y, :, 4 * g + li],
                            start=True, stop=True,
                            tile_position=(0, 32 * li),
                        )
                for g2 in range(4):
                    nc.scalar.activation(
                        P1[0:32, 2 * yb:2 * yb + 2, :, g2::4]
                          .rearrange("p y b x -> p y x b"),
                        PS[32 * g2:32 * g2 + 32, :, :, :],
                        AF.Prelu, alpha=ALPHA,
                    )
                # x-shifted replicas for partition blocks 1, 2
                nc.vector.tensor_copy(
                    P1[32:64, 2 * yb:2 * yb + 2, :, 0:31],
                    P1[0:32, 2 * yb:2 * yb + 2, :, 1:32])
                nc.vector.tensor_copy(
                    P1[64:96, 2 * yb:2 * yb + 2, :, 0:30],
                    P1[0:32, 2 * yb:2 * yb + 2, :, 2:32])

            # ---- LC2: 6 local rows, 28 x-locs (7 groups exactly) ----
            for y in range(6):
                W2t = wpool.tile([97, 7, 3, 128], bf, tag="wrow")
                nc.sync.dma_start(W2t[:], w2p_d[y])
                PS = pspool.tile([128, 7, 32], f32, tag="ps")
                for g in range(7):
                    for ky in range(3):
                        for li in range(4):
                            nc.tensor.matmul(
                                PS[32 * li:32 * li + 32, g, :],
                                W2t[:, g, ky, 32 * li:32 * li + 32],
                                P1[:, y + ky, :, 4 * g + li],
                                start=(ky == 0), stop=(ky == 2),
                                tile_position=(0, 32 * li),
                            )
                for g2 in range(4):
                    nc.scalar.activation(
                        P2[0:32, y, :, g2:g2 + 25:4].rearrange("p b x -> p x b"),
                        PS[32 * g2:32 * g2 + 32, :, :],
                        AF.Prelu, alpha=ALPHA,
                    )
                nc.vector.tensor_copy(P2[32:64, y, :, 0:28], P2[0:32, y, :, 1:29])
                nc.vector.tensor_copy(P2[64:96, y, :, 0:28], P2[0:32, y, :, 2:30])

            # ---- LC3: 4 local rows, 28 x-locs (26 true + 2 zero-padded) ----
            for y in range(4):
                W3t = wpool.tile([97, 7, 3, 128], bf, tag="wrow")
                nc.sync.dma_start(W3t[:], w3p_d[y])
                PS = pspool.tile([128, 7, 32], f32, tag="ps")
                for g in range(7):
                    for ky in range(3):
                        for li in range(4):
                            nc.tensor.matmul(
                                PS[32 * li:32 * li + 32, g, :],
                                W3t[:, g, ky, 32 * li:32 * li + 32],
                                P2[:, y + ky, :, 4 * g + li],
                                start=(ky == 0), stop=(ky == 2),
                                tile_position=(0, 32 * li),
                            )
                for g2 in range(4):
                    nc.scalar.activation(
                        ACT3[0:32, :, :, y, g2::4].rearrange("p j b x -> p x j b"),
                        PS[32 * g2:32 * g2 + 32, :, :],
                        AF.Prelu, alpha=ALPHA,
                    )

            # ============== transition: AllToAll to batch sharding ===========
            # conv1 input patches: [kx*32+c, b, ypad28, xpad28]; zero only the
            # 1px border of block 0, interior is overwritten
            PB1 = apool.tile([96, 4, 28, 28], bf, tag="P1")
            nc.vector.memset(PB1[0:32, :, 0:1, :], 0.0)
            nc.vector.memset(PB1[0:32, :, 27:28, :], 0.0)
            nc.vector.memset(PB1[0:32, :, 1:27, 0:1], 0.0)
            nc.vector.memset(PB1[0:32, :, 1:27, 27:28], 0.0)
            if USE_RDMA_A2A:
                # repack to 128 partitions (32*b + och); descriptors were
                # prepped above, data is read when the trigger fires
                for b in range(4):
                    nc.sync.dma_start(A2S[32 * b:32 * b + 32, :, :, :],
                                      ACT3[:, :, b, :, :])
                nc.gpsimd.trigger_dma(count=None, queue_num=0,
                                      signals_writable=[A2S[:], PBQ2[:]])
                for b in range(4):
                    cp = nc.vector.tensor_copy(
                        PB1[0:32, b, 1:27, 1:27],
                        PBQ2[32 * b:32 * b + 32, :, 0:26])
                    cp.wait_op(a2a_rsem, wreg, "sem-ge")
                rel = nc.vector.sem_inc(q0free, 1)
                rel.wait_op(a2a_rsem, wreg, "sem-ge")
            else:
                a2a_in = dpool.tile([8, 32, 4, 4, 28], bf, tag="a2a_in")
                a2a_out = dpool.tile([8, 32, 4, 4, 28], bf, tag="a2a_out")
                nc.gpsimd.dma_start(
                    a2a_in[:].rearrange("j o b y x -> o j (b y x)"),
                    ACT3[:].rearrange("p j b y x -> p j (b y x)"))
                nc.gpsimd.collective_compute(
                    "AllToAll", ALU.bypass, replica_groups=[CORES],
                    ins=[a2a_in.opt()], outs=[a2a_out.opt()])
                # one bulk DMA for the whole A2A result, then cheap DVE
                # row-selection copies (8 small DMAs each pay ~1.5us of DGE
                # + semaphore latency, serializing ~15us after the A2A)
                PBA = apool.tile([32, 8, 4, 4, 28], bf, tag="ACT3")
                nc.gpsimd.dma_start(
                    PBA[:], a2a_out[:].rearrange("i o b y x -> o i b y x"))
                for i in range(NCORES):
                    lo, hi = TAKE[i]
                    nc.vector.tensor_copy(
                        PB1[0:32, :, 1 + lo:1 + hi, 1:27],
                        PBA[:, i, :, lo - ST[i]:hi - ST[i], 0:26])
            nc.vector.tensor_copy(PB1[32:64, :, :, 0:27], PB1[0:32, :, :, 1:28])
            nc.vector.tensor_copy(PB1[64:96, :, :, 0:26], PB1[0:32, :, :, 2:28])
            if USE_RDMA_A2A:
                emit_stat_preps(4)

            # eps tile for sqrt(var + eps)
            EPST = spool.tile([128, 1], f32, tag="EPST")
            nc.vector.memset(EPST[:], EPS)

            def bn_scale_shift(tag, C, TTs, TTq, bn_g, bn_b, n_elems):
                """scale/shift from total sum TTs / sumsq TTq ([C,1] views)."""
                MEAN = spool.tile([C, 1], f32, tag=f"MEAN{tag}")
                MSQ = spool.tile([C, 1], f32, tag=f"MSQ{tag}")
                VAR = spool.tile([C, 1], f32, tag=f"VAR{tag}")
                SD = spool.tile([C, 1], f32, tag=f"SD{tag}")
                SC = spool.tile([C, 1], f32, tag=f"SC{tag}")
                TB = spool.tile([C, 1], f32, tag=f"TB{tag}")
                nc.scalar.mul(MEAN[:], TTs, 1.0 / n_elems)
                nc.scalar.mul(MSQ[:], TTq, 1.0 / n_elems)
                nc.vector.tensor_mul(VAR[:], MEAN[:], MEAN[:])
                nc.vector.tensor_sub(VAR[:], MSQ[:], VAR[:])
                nc.scalar.activation(SD[:], VAR[:], AF.Sqrt, bias=EPST[0:C, :])
                nc.vector.reciprocal(SD[:], SD[:])
                nc.vector.tensor_mul(SC[:], bn_g, SD[:])
                nc.vector.tensor_mul(TB[:], MEAN[:], SC[:])
                nc.vector.tensor_sub(TB[:], bn_b, TB[:])
                return SC, TB

            # ======================= conv1 + BN + pool =======================
            W1B = cpool.tile([96, 3, 64], bf, tag="W1B")
            nc.sync.dma_start(W1B[:], w1b_d[:])
            BN1 = cpool.tile([64, 2], f32, tag="BN1")
            nc.sync.dma_start(BN1[:], bn1_d[:])
            O1 = apool.tile([64, 4, 2, 13, 26], f32, tag="P2")  # (b, yh, y13, x26)
            SA1 = spool.tile([64, 8], f32, tag="SA1")
            QA1 = spool.tile([64, 8], f32, tag="QA1")
            for nb in range(8):
                b, yh = nb // 2, nb % 2
                PS = pspool.tile([64, 13, 26], f32, tag="ps")
                for ky in range(3):
                    nc.tensor.matmul(
                        PS[:], W1B[:, ky, :],
                        PB1[0:96, b, 13 * yh + ky:13 * yh + ky + 13, 0:26],
                        start=(ky == 0), stop=(ky == 2))
                nc.scalar.activation(O1[:, b, yh, :, :], PS[:], AF.Copy,
                                     accum_out=SA1[:, nb:nb + 1])
                SCR = scrpool.tile([64, 13, 26], f32, tag="scr")
                nc.scalar.activation(SCR[:], O1[:, b, yh, :, :], AF.Square,
                                     accum_out=QA1[:, nb:nb + 1])
            nc.vector.memset(SND[1][64:128, :], 0.0)
            nc.vector.tensor_reduce(SND[1][0:64, 0:1], SA1[:],
                                    mybir.AxisListType.X, ALU.add)
            nc.vector.tensor_reduce(SND[1][0:64, 1:2], QA1[:],
                                    mybir.AxisListType.X, ALU.add)
            TT1 = allreduce_stats(1)
            SC1, TB1 = bn_scale_shift("1", 64, TT1[0:64, 0:1], TT1[0:64, 1:2],
                                      BN1[:, 0:1], BN1[:, 1:2], N1)
            PA = apool.tile([64, 4, 26, 13], f32, tag="PA")
            T1 = O1[:].rearrange("p b h y x -> p b (h y) x")
            for bh in range(2):
                nc.scalar.activation(O1[:, 2 * bh:2 * bh + 2], 
                                     O1[:, 2 * bh:2 * bh + 2], AF.Prelu,
                                     bias=TB1[:], scale=SC1[:], alpha=ALPHA)
                nc.vector.tensor_max(PA[:, 2 * bh:2 * bh + 2],
                                     T1[:, 2 * bh:2 * bh + 2, :, 0::2],
                                     T1[:, 2 * bh:2 * bh + 2, :, 1::2])
            PB2a = apool.tile([128, 4, 15, 15], bf, tag="ACT3")
            PB2b = apool.tile([64, 4, 15, 15], bf, tag="PB2b")
            nc.vector.memset(PB2a[0:64, :, 0:1, :], 0.0)
            nc.vector.memset(PB2a[0:64, :, 14:15, :], 0.0)
            nc.vector.memset(PB2a[0:64, :, 1:14, 0:1], 0.0)
            nc.vector.memset(PB2a[0:64, :, 1:14, 14:15], 0.0)
            nc.vector.tensor_max(PB2a[0:64, :, 1:14, 1:14],
                                 PA[:, :, 0:26:2, :], PA[:, :, 1:26:2, :])
            nc.vector.tensor_copy(PB2a[64:128, :, :, 0:14], PB2a[0:64, :, :, 1:15])
            nc.vector.tensor_copy(PB2b[0:64, :, :, 0:13], PB2a[0:64, :, :, 2:15])

            # ======================= conv2 + BN + pool =======================
            W2BA = cpool.tile([128, 3, 128], bf, tag="W2BA")
            nc.sync.dma_start(W2BA[:], w2ba_d[:])
            W2BB = cpool.tile([64, 3, 128], bf, tag="W2BB")
            nc.sync.dma_start(W2BB[:], w2bb_d[:])
            BN2 = cpool.tile([128, 2], f32, tag="BN2")
            nc.sync.dma_start(BN2[:], bn2_d[:])
            O2 = apool.tile([128, 4, 13, 13], f32, tag="O2")
            SA2 = spool.tile([128, 4], f32, tag="SA2")
            QA2 = spool.tile([128, 4], f32, tag="QA2")
            for b in range(4):
                PS = pspool.tile([128, 13, 13], f32, tag="ps")
                for ky in range(3):
                    nc.tensor.matmul(PS[:], W2BA[:, ky, :],
                                     PB2a[:, b, ky:ky + 13, 0:13],
                                     start=(ky == 0), stop=False)
                for ky in range(3):
                    nc.tensor.matmul(PS[:], W2BB[:, ky, :],
                                     PB2b[:, b, ky:ky + 13, 0:13],
                                     start=False, stop=(ky == 2))
                nc.scalar.activation(O2[:, b, :, :], PS[:], AF.Copy,
                                     accum_out=SA2[:, b:b + 1])
                SCR = scrpool.tile([128, 13, 13], f32, tag="scr")
                nc.scalar.activation(SCR[:], O2[:, b, :, :], AF.Square,
                                     accum_out=QA2[:, b:b + 1])
            nc.vector.tensor_reduce(SND[2][:, 0:1], SA2[:],
                                    mybir.AxisListType.X, ALU.add)
            nc.vector.tensor_reduce(SND[2][:, 1:2], QA2[:],
                                    mybir.AxisListType.X, ALU.add)
            TT2 = allreduce_stats(2)
            SC2, TB2 = bn_scale_shift("2", 128, TT2[:, 0:1], TT2[:, 1:2],
                                      BN2[:, 0:1], BN2[:, 1:2], N2)
            PA2 = apool.tile([128, 4, 12, 6], f32, tag="PA2")
            for bh in range(2):
                nc.scalar.activation(O2[:, 2 * bh:2 * bh + 2],
                                     O2[:, 2 * bh:2 * bh + 2], AF.Prelu,
                                     bias=TB2[:], scale=SC2[:], alpha=ALPHA)
                nc.vector.tensor_max(PA2[:, 2 * bh:2 * bh + 2],
                                     O2[:, 2 * bh:2 * bh + 2, 0:12, 0:12:2],
                                     O2[:, 2 * bh:2 * bh + 2, 0:12, 1:13:2])
            PB3a = apool.tile([128, 4, 8, 8], bf, tag="P1")
            PB3b = apool.tile([128, 4, 8, 8], bf, tag="PB3b")
            PB3c = apool.tile([128, 4, 8, 8], bf, tag="PB3c")
            nc.vector.memset(PB3a[:, :, 0:1, :], 0.0)
            nc.vector.memset(PB3a[:, :, 7:8, :], 0.0)
            nc.vector.memset(PB3a[:, :, 1:7, 0:1], 0.0)
            nc.vector.memset(PB3a[:, :, 1:7, 7:8], 0.0)
            nc.vector.tensor_max(PB3a[:, :, 1:7, 1:7],
                                 PA2[:, :, 0:12:2, :], PA2[:, :, 1:12:2, :])
            nc.vector.tensor_copy(PB3b[:, :, :, 0:7], PB3a[:, :, :, 1:8])
            nc.vector.tensor_copy(PB3c[:, :, :, 0:6], PB3a[:, :, :, 2:8])

            # ======================= conv3 + BN + pool =======================
            W3B = cpool.tile([128, 3, 3, 256], bf, tag="W3B")
            nc.sync.dma_start(W3B[:], w3b_d[:])
            BN3 = cpool.tile([128, 2, 2], f32, tag="BN3")
            nc.sync.dma_start(BN3[:], bn3_d[:])
            O3 = apool.tile([128, 2, 4, 6, 6], f32, tag="O3")  # (mh, b, y, x)
            SA3 = spool.tile([128, 2], f32, tag="SA3")
            QA3 = spool.tile([128, 2], f32, tag="QA3")
            PBs = [PB3a, PB3b, PB3c]
            for mh in range(2):
                PS = pspool.tile([128, 4, 6, 6], f32, tag="ps")
                for ky in range(3):
                    for kx in range(3):
                        nc.tensor.matmul(
                            PS[:], W3B[:, ky, kx, 128 * mh:128 * mh + 128],
                            PBs[kx][:, :, ky:ky + 6, 0:6],
                            start=(ky == 0 and kx == 0), stop=(ky == 2 and kx == 2))
                nc.scalar.activation(O3[:, mh, :, :, :], PS[:], AF.Copy,
                                     accum_out=SA3[:, mh:mh + 1])
                SCR = scrpool.tile([128, 4, 6, 6], f32, tag="scr")
                nc.scalar.activation(SCR[:], O3[:, mh, :, :, :], AF.Square,
                                     accum_out=QA3[:, mh:mh + 1])
            nc.vector.tensor_copy(SND[3][:, 0:4:2], SA3[:])
            nc.vector.tensor_copy(SND[3][:, 1:4:2], QA3[:])
            TT3 = allreduce_stats(3)
            SC3, TB3 = {}, {}
            for mh in range(2):
                SC3[mh], TB3[mh] = bn_scale_shift(
                    f"3{mh}", 128, TT3[:, 2 * mh:2 * mh + 1],
                    TT3[:, 2 * mh + 1:2 * mh + 2],
                    BN3[:, mh, 0:1], BN3[:, mh, 1:2], N3)
            PB4 = []
            for kx in range(3):
                row = []
                for mh in range(2):
                    pb4t = apool.tile([128, 4, 5, 5], bf, tag=f"PB4{kx}{mh}")
                    row.append(pb4t)
                PB4.append(row)
            for mh in range(2):
                nc.scalar.activation(O3[:, mh, :, :, :], O3[:, mh, :, :, :],
                                     AF.Prelu, bias=TB3[mh][:], scale=SC3[mh][:],
                                     alpha=ALPHA)
                T3 = O3[:, mh, :, :, :]
                PA3 = apool.tile([128, 4, 6, 3], f32, tag=f"PA3{mh}")
                nc.vector.tensor_max(PA3[:], T3[:, :, :, 0::2], T3[:, :, :, 1::2])
                nc.vector.memset(PB4[0][mh][:, :, 0:1, :], 0.0)
                nc.vector.memset(PB4[0][mh][:, :, 4:5, :], 0.0)
                nc.vector.memset(PB4[0][mh][:, :, 1:4, 0:1], 0.0)
                nc.vector.memset(PB4[0][mh][:, :, 1:4, 4:5], 0.0)
                nc.vector.tensor_max(PB4[0][mh][:, :, 1:4, 1:4],
                                     PA3[:, :, 0:6:2, :], PA3[:, :, 1:6:2, :])
                nc.vector.tensor_copy(PB4[1][mh][:, :, :, 0:4],
                                      PB4[0][mh][:, :, :, 1:5])
                nc.vector.tensor_copy(PB4[2][mh][:, :, :, 0:3],
                                      PB4[0][mh][:, :, :, 2:5])

            # ======================= convf + BN + tanh =======================
            WFB = cpool.tile([128, 2, 3, 3, 256], bf, tag="WFB")
            nc.sync.dma_start(WFB[:], wfb_d[:])
            BNF = cpool.tile([128, 2, 2], f32, tag="BNF")
            nc.sync.dma_start(BNF[:], bnf_d[:])
            OF = apool.tile([128, 2, 4, 3, 3], f32, tag="OF")
            SAF = spool.tile([128, 2], f32, tag="SAF")
            QAF = spool.tile([128, 2], f32, tag="QAF")
            for mh in range(2):
                PS = pspool.tile([128, 4, 3, 3], f32, tag="ps")
                first = True
                for cb in range(2):
                    for ky in range(3):
                        for kx in range(3):
                            nc.tensor.matmul(
                                PS[:], WFB[:, cb, ky, kx, 128 * mh:128 * mh + 128],
                                PB4[kx][cb][:, :, ky:ky + 3, 0:3],
                                start=first, stop=(cb == 1 and ky == 2 and kx == 2))
                            first = False
                nc.scalar.activation(OF[:, mh, :, :, :], PS[:], AF.Copy,
                                     accum_out=SAF[:, mh:mh + 1])
                SCR = scrpool.tile([128, 4, 3, 3], f32, tag="scr")
                nc.scalar.activation(SCR[:], OF[:, mh, :, :, :], AF.Square,
                                     accum_out=QAF[:, mh:mh + 1])
            nc.vector.tensor_copy(SND[4][:, 0:4:2], SAF[:])
            nc.vector.tensor_copy(SND[4][:, 1:4:2], QAF[:])
            TTF = allreduce_stats(4)
            for mh in range(2):
                SCt, TBt = bn_scale_shift(
                    f"f{mh}", 128, TTF[:, 2 * mh:2 * mh + 1],
                    TTF[:, 2 * mh + 1:2 * mh + 2],
                    BNF[:, mh, 0:1], BNF[:, mh, 1:2], NF)
                OUTT = apool.tile([128, 4, 3, 3], bf, tag=f"OUTT{mh}")
                nc.scalar.activation(OUTT[:], OF[:, mh, :, :, :], AF.Tanh,
                                     bias=TBt[:], scale=SCt[:])
                nc.sync.dma_start(
                    out_d[:, 128 * mh:128 * mh + 128, :, :]
                        .rearrange("b c y x -> c b y x"),
                    OUTT[:])
    return nc


def _prep(inputs):
    """Host-side shard + layout prep. Pure data movement (plus dtype cast)."""
    f32 = np.float32
    x = np.asarray(inputs["x"], f32)
    lc1_w = np.asarray(inputs["lc1_w"], f32)[0]  # (32,3,30,30,9)
    lc1_b = np.asarray(inputs["lc1_b"], f32)[0]  # (32,30,30)
    lc2_w = np.asarray(inputs["lc2_w"], f32)[0]  # (32,32,28,28,9)
    lc2_b = np.asarray(inputs["lc2_b"], f32)[0]
    lc3_w = np.asarray(inputs["lc3_w"], f32)[0]  # (32,32,26,26,9)
    lc3_b = np.asarray(inputs["lc3_b"], f32)[0]

    # replicated stage-B weights
    c1w = np.asarray(inputs["c1_w"], f32)
    c2w = np.asarray(inputs["c2_w"], f32)
    c3w = np.asarray(inputs["c3_w"], f32)
    cfw = np.asarray(inputs["cf_w"], f32)
    w1b = np.ascontiguousarray(c1w.transpose(3, 1, 2, 0).reshape(96, 3, 64)).astype(CAST)
    w2ba = np.ascontiguousarray(
        c2w[:, :, :, 0:2].transpose(3, 1, 2, 0).reshape(128, 3, 128)).astype(CAST)
    w2bb = np.ascontiguousarray(c2w[:, :, :, 2].transpose(1, 2, 0)).astype(CAST)
    w3b = np.ascontiguousarray(c3w.transpose(1, 2, 3, 0)).astype(CAST)
    wfb = np.ascontiguousarray(
        cfw.reshape(256, 2, 128, 3, 3).transpose(2, 1, 3, 4, 0)).astype(CAST)
    bn1 = np.stack([np.asarray(inputs["c1_g"], f32),
                    np.asarray(inputs["c1_beta"], f32)], axis=1)
    bn2 = np.stack([np.asarray(inputs["c2_g"], f32),
                    np.asarray(inputs["c2_beta"], f32)], axis=1)
    bn3 = np.stack([np.asarray(inputs["c3_g"], f32).reshape(2, 128).T,
                    np.asarray(inputs["c3_beta"], f32).reshape(2, 128).T], axis=2)
    bnf = np.stack([np.asarray(inputs["cf_g"], f32).reshape(2, 128).T,
                    np.asarray(inputs["cf_beta"], f32).reshape(2, 128).T], axis=2)

    def lc_pack(wsl, bsl, nrow, width):
        """wsl: (32o,32c,nrow,width,9) -> (nrow, 97, G, 3, 128); bsl: (32o,nrow,width)"""
        G = 7
        wp = np.zeros((32, 32, nrow, 4 * G, 9), f32)
        wp[:, :, :, :width] = wsl
        bp = np.zeros((32, nrow, 4 * G), f32)
        bp[:, :, :width] = bsl
        arr = wp.reshape(32, 32, nrow, G, 4, 3, 3)  # o c y g li ky kx
        arr = arr.transpose(2, 6, 1, 3, 5, 4, 0).reshape(nrow, 96, G, 3, 128)
        outw = np.zeros((nrow, 97, G, 3, 128), f32)
        outw[:, :96] = arr
        outw[:, 96, :, 0, :] = bp.transpose(1, 2, 0).reshape(nrow, G, 4, 32)\
                                 .reshape(nrow, G, 128)
        return outw.astype(CAST)

    # core c computes LC3 rows ST[lab[c]] and stage-B images [4*lab[c]..);
    # with the rdma A2A, lab is the XOR label from discovery, else identity
    lab = _discover_dmap() if USE_RDMA_A2A else list(range(NCORES))

    in_maps = []
    xpad = np.zeros((32, 3, 32, 34), f32)
    xpad[:, :, :, :32] = x
    for c in range(NCORES):
        s = ST[lab[c]]
        if USE_RDMA_A2A:
            # batch-group permutation: XP group m holds image group lab^m,
            # so A2A slot m always carries the receiver's own images
            border = np.concatenate(
                [np.arange(4 * (lab[c] ^ m), 4 * (lab[c] ^ m) + 4)
                 for m in range(8)])
        else:
            border = np.arange(32)
        xb = xpad[border]
        xp = np.zeros((28, 8, 32, 32), f32)
        for ky in range(3):
            for kx in range(3):
                k = ky * 3 + kx
                blk = xb[:, :, s + ky:s + ky + 8, kx:kx + 32]  # (b,c,y,x)
                xp[3 * k:3 * k + 3] = blk.transpose(1, 2, 0, 3)
        xp[27] = 1.0

        w1sl = np.zeros((32, 3, 8, 32, 9), f32)
        w1sl[:, :, :, :30] = lc1_w[:, :, s:s + 8]
        b1sl = np.zeros((32, 8, 32), f32)
        b1sl[:, :, :30] = lc1_b[:, s:s + 8]
        arr = w1sl.reshape(32, 3, 8, 8, 4, 9)  # o c y g li k
        arr = arr.transpose(5, 1, 2, 3, 4, 0).reshape(27, 8, 8, 128)
        w1p = np.zeros((28, 8, 8, 128), f32)
        w1p[:27] = arr
        w1p[27] = b1sl.transpose(1, 2, 0).reshape(8, 8, 4, 32).reshape(8, 8, 128)

        w2p = lc_pack(lc2_w[:, :, s:s + 6], lc2_b[:, s:s + 6], 6, 28)
        w3p = lc_pack(lc3_w[:, :, s:s + 4], lc3_b[:, s:s + 4], 4, 26)

        in_maps.append({
            "xp": xp.astype(CAST), "w1p": w1p.astype(CAST),
            "w2p": w2p, "w3p": w3p,
            "w1b": w1b, "w2ba": w2ba, "w2bb": w2bb, "w3b": w3b, "wfb": wfb,
            "bn1": bn1, "bn2": bn2, "bn3": bn3, "bnf": bnf,
            "myrow": np.array([[ST[lab[c]] * 28]], np.uint32),
        })
    return in_maps


def get_nc():
    if "nc" not in _cache:
        nc = _build()
        nc.compile()
        _cache["nc"] = nc
    return _cache["nc"]


class Runner:
    """Persistent SPMD executor for the compiled NEFF.

    run_bass_kernel_spmd rebuilds its jit closure (retrace + relower,
    ~0.6s) and re-transfers every input (~60MB over the axon tunnel,
    ~1.1s) on EVERY call. This runner builds the sharded jit ONCE,
    uploads inputs ONCE (device-resident; one batched device_put), and
    then each execution is just dispatch + output fetch. The NEFF's
    donated-output workaround is replaced by persistent dummy output
    buffers: the kernel writes every element of `out`, so the zero-init
    (only needed for unwritten bytes) is unnecessary, and without jit
    donation the buffers survive across calls (verified bit-identical
    against the run_bass_kernel_spmd path).
    """

    def __init__(self, nc):
        import jax
        from jax.sharding import Mesh, PartitionSpec, NamedSharding
        from jax.experimental.shard_map import shard_map

        self.jax = jax
        bass2jax.install_neuronx_cc_hook()
        assert nc.dbg_addr is None or not nc.dbg_callbacks
        pname = nc.partition_id_tensor.name if nc.partition_id_tensor else None
        in_names, out_names, out_avals = [], [], []
        for alloc in nc.m.functions[0].allocations:
            if not isinstance(alloc, mybir.MemoryLocationSet):
                continue
            name = alloc.memorylocations[0].name
            if alloc.kind == "ExternalInput":
                if name != pname:
                    in_names.append(name)
            elif alloc.kind == "ExternalOutput":
                shape = tuple(alloc.tensor_shape)
                dtype = mybir.dt.np(alloc.dtype)
                out_names.append(name)
                out_avals.append(jax.core.ShapedArray(shape, dtype))
        self.in_names, self.out_names, self.out_avals = in_names, out_names, out_avals
        in_names_all = in_names + out_names + ([pname] if pname else [])

        def _body(*args):
            operands = list(args)
            if pname is not None:
                operands.append(bass2jax.partition_id_tensor())
            return tuple(bass2jax._bass_exec_p.bind(
                *operands, out_avals=tuple(out_avals),
                in_names=tuple(in_names_all), out_names=tuple(out_names),
                lowering_input_output_aliases=(), sim_require_finite=True,
                sim_require_nnan=True, nc=nc))

        devices = jax.devices()[:NCORES]
        assert len(devices) == NCORES
        mesh = Mesh(np.asarray(devices), ("core",))
        self.sharding = NamedSharding(mesh, PartitionSpec("core"))
        n_all = len(in_names) + len(out_names)
        self.fn = jax.jit(
            shard_map(_body, mesh=mesh,
                      in_specs=(PartitionSpec("core"),) * n_all,
                      out_specs=(PartitionSpec("core"),) * len(out_names),
                      check_rep=False),
            keep_unused=True)
        # persistent (non-donated) buffers for the NEFF output slots
        self.out_bufs = jax.device_put(
            [np.zeros((NCORES * av.shape[0], *av.shape[1:]), av.dtype)
             for av in out_avals], self.sharding)

    def upload(self, in_maps):
        """Concat per-core inputs and transfer once; returns device args."""
        concat = [np.concatenate([np.asarray(in_maps[c][nm])
                                  for c in range(NCORES)], axis=0)
                  for nm in self.in_names]
        dev_in = self.jax.device_put(concat, self.sharding)
        self.jax.block_until_ready(dev_in)
        return dev_in

    def submit(self, dev_in):
        """Dispatch one SPMD execution (async); returns output handles."""
        return self.fn(*dev_in, *self.out_bufs)

    def fetch(self, handles):
        """Materialize one execution's outputs to numpy (full shape)."""
        for h in handles:
            h.copy_to_host_async()
        return [np.asarray(h) for h in handles]

    def fetch_all(self, handle_list):
        """Materialize many executions' outputs; starts every D2H copy
        before blocking on any (one latency for the whole batch)."""
        for handles in handle_list:
            for h in handles:
                h.copy_to_host_async()
        return [[np.asarray(h) for h in handles] for handles in handle_list]


def get_runner():
    if "runner" not in _cache:
        _cache["runner"] = Runner(get_nc())
    return _cache["runner"]


def kernel(**inputs) -> np.ndarray:
    r = get_runner()
    in_maps = _prep(inputs)
    lab = _discover_dmap() if USE_RDMA_A2A else list(range(NCORES))
    dev_in = r.upload(in_maps)
    res = r.fetch(r.submit(dev_in))[0]  # (NCORES*4, 256, 3, 3)
    out = np.empty((32, 256, 3, 3), np.float32)
    for c in range(NCORES):
        out[4 * lab[c]:4 * lab[c] + 4] = res[4 * c:4 * c + 4].astype(np.float32)
    return out

